# revision 1
# baseline (speedup 1.0000x reference)
"""Trainium2 Bass kernel for nn_Attention (pre-LN causal attention block).

Reference computation (B=2, T=2048, C=1024, H=16, D=64, fp32):
    xn = LayerNorm(x)                       (eps=1e-6)
    qkv = xn @ qkv_w + qkv_b;  q,k,v = split(qkv)
    scores = (q @ k^T) / sqrt(D), causal-masked, softmax
    out = (softmax @ v) reshaped @ proj_w + proj_b

Sharding (8 cores): data-parallel over B (cores 0-3 <- batch 0, 4-7 <- batch 1)
x tensor-parallel over heads (4 heads/core: qkv_w column-sharded, proj_w
row-sharded).  Each core emits a partial projection output; the host sums the
4 partials per batch and adds proj_b (the "all-reduce after proj" done host-side).

Device kernel design notes:
  - All matmuls run as float32r (TF32-class, full PE rate at N>=256; measured
    rel-err ~1.8e-4 per stage on HW).
  - Scores are computed TRANSPOSED (s^T[k,q] = K @ Q^T) so that the PV matmul
    consumes softmax tiles directly (lhsT = token-major V) with no transposes.
  - Softmax: scores here are O(1), so exp() without max-subtraction is exact
    enough; the denominator comes free by augmenting V with a ones column
    (row 64 of the PV psum accumulates sum(p)).  Masking is a multiplicative
    0/1 mask applied to p=exp(s) (identical to the reference's -inf where()).
  - x^T (needed as matmul contraction input) is produced by an exact bf16
    hi/lo split of LN(x), one batched 2-byte DMA-xbar transpose per token
    tile ([128,2048]bf16 -> [128,16,128]), and a hi+lo add.
  - Causal structure skips fully-masked k/q tile combinations (~40% of
    attention FLOPs + exp work).
"""

import os

import numpy as np

import concourse.bass as bass
import concourse.tile as tile
from concourse import mybir
from concourse.bass_utils import run_bass_kernel_spmd

LAST_RESULT = None
F32 = mybir.dt.float32
F32R = mybir.dt.float32r
BF16 = mybir.dt.bfloat16

B, T, C = 2, 2048, 1024
H, D = 16, 64
HL = 4            # heads per core
CL = HL * D       # local c-dim (256)
NT = T // 128     # 16 token tiles
NCC = C // 128    # 8 contraction chunks
LN_EPS = 1e-6
SCORE_SCALE = 0.125  # 1/sqrt(D)/TEMP


def _split_waits(nc, limit=1):
    """This container's walrus rejects instructions with >1 sem wait ("Too many
    sync wait commands").  Move excess waits onto same-engine NOPs inserted
    just before the instruction (equivalent under per-engine program order)."""
    n = 0
    for f in nc.m.functions:
        for b in f.blocks:
            insts = b.instructions
            if not any(
                i.sync_info is not None and len(i.sync_info.on_wait) > limit
                for i in insts
            ):
                continue
            new = []
            for inst in insts:
                si = inst.sync_info
                if si is not None and len(si.on_wait) > limit:
                    waits = list(si.on_wait)
                    excess, keep = waits[:-limit], waits[-limit:]
                    for j in range(0, len(excess), limit):
                        n += 1
                        nop = mybir.InstNoOp(name=f"I-wsplit-{n}", ins=[], outs=[])
                        nop.engine = inst.engine
                        nop.sync_info = mybir.SyncInfo(
                            on_wait=excess[j : j + limit], on_update=[]
                        )
                        new.append(nop)
                    inst.sync_info = mybir.SyncInfo(
                        on_wait=keep, on_update=list(si.on_update)
                    )
                new.append(inst)
            b.instructions = new
    return n


def _build(mode, vbias):
    """mode: 'causal' (tile-skip + diag mask), 'none' (no mask), 'full'
    (arbitrary mask, maskT input).  vbias: apply per-column v bias."""
    nc = bass.Bass(name="attnblk")
    x_in = nc.declare_dram_parameter("x_b", [T, C], F32, isOutput=False)
    wqkv = nc.declare_dram_parameter("wqkv", [C, 3 * CL], F32R, isOutput=False)
    bqkv = nc.declare_dram_parameter("bqkv", [3 * CL], F32, isOutput=False)
    wp = nc.declare_dram_parameter("wp", [CL, C], F32R, isOutput=False)
    if mode == "causal":
        maskd = nc.declare_dram_parameter("maskd", [128, 128], F32R, isOutput=False)
    elif mode == "full":
        maskt = nc.declare_dram_parameter("maskt", [T, T], F32R, isOutput=False)
    y_out = nc.declare_dram_parameter("y_part", [T, C], F32, isOutput=True)

    scratch_sums = nc.dram_tensor("scratch_sums", [HL, T], F32)

    Exp = mybir.ActivationFunctionType.Exp
    Sqrt = mybir.ActivationFunctionType.Sqrt
    Ident = mybir.ActivationFunctionType.Identity
    MULT = mybir.AluOpType.mult
    SUB = mybir.AluOpType.subtract
    ADD = mybir.AluOpType.add

    with tile.TileContext(nc) as tc:
        with (
            tc.tile_pool(name="persist", bufs=1) as pp,
            tc.tile_pool(name="ps_small", bufs=1) as pco,
        ):
            # ---- persistent sbuf tensors
            qT = pp.tile([128, 2, T], F32R, tag="qT")       # pair-stacked Q^T
            kT = pp.tile([128, 2, T], F32R, tag="kT")
            Vp = pp.tile([128, NT, HL, D + 1], F32R, tag="Vp")  # V' per head
            w_p = pp.tile([128, 2, C], F32R, tag="w_p")
            eps_t = pco.tile([128, 1], F32, tag="eps")
            bq_t = pco.tile([128, 2, 1], F32, tag="bq")
            bk_t = pco.tile([128, 2, 1], F32, tag="bk")

            nc.vector.memset(eps_t, LN_EPS)
            nc.sync.dma_start(
                out=w_p, in_=wp.ap().rearrange("(P p) n -> p P n", p=128)
            )
            nc.sync.dma_start(
                out=bq_t, in_=bqkv.ap()[0:CL].rearrange("(P p) -> p P", p=128)
            )
            nc.sync.dma_start(
                out=bk_t,
                in_=bqkv.ap()[CL : 2 * CL].rearrange("(P p) -> p P", p=128),
            )
            # fill V' with ones; the V drains overwrite cols 0..D-1, leaving
            # each head's ones column (col D) in place
            nc.vector.memset(Vp[:, :, :, :].bitcast(F32), 1.0)

            with tc.tile_pool(name="wq_pool", bufs=1) as wqp:
                w_qkv = wqp.tile([128, NCC, 3 * CL], F32R, tag="wqkv")
                nc.sync.dma_start(
                    out=w_qkv, in_=wqkv.ap().rearrange("(cc p) n -> p cc n", p=128)
                )
                if vbias:
                    bv_t = wqp.tile([128, CL], F32, tag="bv")
                    bv_ap = bass.AP(
                        tensor=bqkv.ap().tensor, offset=2 * CL, ap=[[0, 128], [1, CL]]
                    )
                    nc.sync.dma_start(out=bv_t, in_=bv_ap)

                with tc.tile_pool(name="xnT_pool", bufs=1) as xtp:
                    xnT = xtp.tile([128, NCC, T], F32R, tag="xnT")

                    # ---- Phase B/C: LN + exact bf16 hi/lo split + batched
                    # xbar transpose ([128,1024] -> [128, 8, 128] in one DMA)
                    with (
                        tc.tile_pool(name="ln_pool", bufs=3) as lnp,
                        tc.tile_pool(name="hilo_pool", bufs=4) as hlp,
                        tc.tile_pool(name="tst_pool", bufs=4) as tsp,
                    ):
                        for tt in range(NT):
                            x_t = lnp.tile([128, C], F32, tag="x")
                            nc.sync.dma_start(
                                out=x_t, in_=x_in[tt * 128 : (tt + 1) * 128, :]
                            )
                            stats = lnp.tile([128, 2, 6], F32, tag="stats")
                            xg = x_t[:, :].rearrange("p (g d) -> p g d", g=2)
                            for g in range(2):
                                nc.vector.bn_stats(out=stats[:, g, :], in_=xg[:, g, :])
                            mv = lnp.tile([128, 2], F32, tag="mv")
                            nc.vector.bn_aggr(out=mv[:, :], in_=stats[:, :, :])
                            rstd = lnp.tile([128, 1], F32, tag="rstd")
                            nc.scalar.activation(
                                out=rstd[:, :], in_=mv[:, 1:2], func=Sqrt,
                                bias=eps_t[:, :], scale=1.0,
                            )
                            nc.vector.reciprocal(out=rstd[:, :], in_=rstd[:, :])
                            nmr = lnp.tile([128, 1], F32, tag="nmr")
                            nc.vector.tensor_tensor(
                                out=nmr[:, :], in0=mv[:, 0:1], in1=rstd[:, :], op=MULT
                            )
                            nc.vector.tensor_scalar_mul(nmr[:, :], nmr[:, :], -1.0)
                            xn_t = lnp.tile([128, C], F32, tag="xn")
                            nc.scalar.activation(
                                out=xn_t[:, :], in_=x_t[:, :], func=Ident,
                                bias=nmr[:, :], scale=rstd[:, :],
                            )
                            hl_t = hlp.tile([128, 2 * C], BF16, tag="hl")
                            nc.gpsimd.tensor_copy(out=hl_t[:, 0:C], in_=xn_t[:, :])
                            nc.vector.tensor_tensor(
                                out=hl_t[:, C : 2 * C], in0=xn_t[:, :],
                                in1=hl_t[:, 0:C], op=SUB,
                            )
                            hlT = tsp.tile([128, 2 * NCC, 128], BF16, tag="hlT")
                            eng = nc.sync if tt % 2 == 0 else nc.scalar
                            eng.dma_start(
                                out=hlT[:, :, :], in_=hl_t[:, :], transpose=True
                            )
                            nc.gpsimd.tensor_tensor(
                                out=xnT[:, :, tt * 128 : (tt + 1) * 128],
                                in0=hlT[:, 0:NCC, :], in1=hlT[:, NCC : 2 * NCC, :],
                                op=ADD,
                            )


                    # ---- Phase D: QKV projections
                    with tc.tile_pool(name="qkv_ps", bufs=2, space="PSUM") as qps:
                        def emit_qk(P):
                            for which, dest, bias_t in (
                                (0, qT, bq_t), (1, kT, bk_t),
                            ):
                                wcol = which * CL + P * 128
                                for nt4 in range(4):
                                    ps = qps.tile(
                                        [128, 512], F32, tag="qk",
                                        name=f"qkps_{P}_{which}_{nt4}",
                                    )
                                    for cc in range(NCC):
                                        nc.tensor.matmul(
                                            ps[:, :],
                                            w_qkv[:, cc, wcol : wcol + 128],
                                            xnT[:, cc, nt4 * 512 : (nt4 + 1) * 512],
                                            start=(cc == 0), stop=(cc == NCC - 1),
                                        )
                                    nc.scalar.activation(
                                        out=dest[:, P, nt4 * 512 : (nt4 + 1) * 512],
                                        in_=ps[:, :], func=Ident,
                                        bias=bias_t[:, P, :], scale=1.0,
                                    )
                        emit_qk(0)
                        # V: token-major, all 4 heads at once (N=256)
                        for tt in range(NT):
                            ps = qps.tile([128, CL], F32, tag="v")
                            for cc in range(NCC):
                                nc.tensor.matmul(
                                    ps[:, :],
                                    xnT[:, cc, tt * 128 : (tt + 1) * 128],
                                    w_qkv[:, cc, 2 * CL : 3 * CL],
                                    start=(cc == 0), stop=(cc == NCC - 1),
                                )
                            psh = ps[:, :].rearrange("p (h d) -> p h d", h=HL)
                            if vbias:
                                bvh = bv_t[:, :].rearrange("p (h d) -> p h d", h=HL)
                                nc.vector.tensor_tensor(
                                    out=Vp[:, tt, :, 0:D], in0=psh, in1=bvh, op=ADD
                                )
                            else:
                                nc.scalar.copy(out=Vp[:, tt, :, 0:D], in_=psh)
                        emit_qk(1)

            # ---- Phase E/F: attention per head
            with tc.tile_pool(name="attn_persist", bufs=1) as app:
              attnT = app.tile([128, 2, T], F32R, tag="attnT")
              sums_st = app.tile([128, T], F32, tag="sums")
              recip_bc = app.tile([128, 2, T], F32, tag="recip")
              if mode == "causal":
                  maskd_t = app.tile([128, 128], F32R, tag="maskd")
                  nc.sync.dma_start(out=maskd_t, in_=maskd[:, :])
              with (
                tc.tile_pool(name="pv_ps", bufs=4, space="PSUM") as vps,
                tc.tile_pool(name="sc_ps", bufs=2, space="PSUM") as sps,
                tc.tile_pool(name="p_pool", bufs=6) as ppool,
                tc.tile_pool(name="mask_pool", bufs=2) as mpool,
              ):
                for h in range(HL):
                    P, hp = h // 2, (h % 2) * 64
                    pv = [vps.tile([65, 512], F32, tag="pv", name=f"pv_{h}_{jj}")
                          for jj in range(4)]
                    for i in range(NT):
                        wlo = i // 8 if mode == "causal" else 0
                        ptiles = {}
                        for w in range(wlo, 2):
                            a0 = max(128 * i, 1024 * w) if mode == "causal" else 1024 * w
                            flo = (a0 // 512) * 512
                            ps = sps.tile([128, 1024], F32, tag="sc")
                            for c5 in range(flo, 1024 * (w + 1), 512):
                                nc.tensor.matmul(
                                    ps[:, c5 - 1024 * w : c5 - 1024 * w + 512],
                                    kT[hp : hp + 64, P, i * 128 : (i + 1) * 128],
                                    qT[hp : hp + 64, P, c5 : c5 + 512],
                                    start=True, stop=True,
                                )
                            p_t = ppool.tile([128, 1024], F32R, tag="p")
                            if (
                                mode == "causal" and w == i // 8 and i % 4 != 0
                            ):
                                # zero the PV-visible gap [512*(i//4), 128i)
                                gs = 512 * (i // 4) - 1024 * w
                                ge = 128 * i - 1024 * w
                                nc.vector.memset(p_t[:, gs:ge].bitcast(F32), 0.0)
                            nc.scalar.activation(
                                out=p_t[:, a0 - 1024 * w : 1024],
                                in_=ps[:, a0 - 1024 * w : 1024],
                                func=Exp, scale=SCORE_SCALE,
                            )
                            ptiles[w] = p_t
                            if mode == "full":
                                m_t = mpool.tile([128, 1024], F32R, tag="m")
                                nc.sync.dma_start(
                                    out=m_t,
                                    in_=maskt[
                                        i * 128 : (i + 1) * 128,
                                        1024 * w : 1024 * (w + 1),
                                    ],
                                )
                                nc.vector.tensor_tensor(
                                    out=p_t[:, :], in0=p_t[:, :], in1=m_t[:, :],
                                    op=MULT,
                                )
                        if mode == "causal":
                            w0 = i // 8
                            off = 128 * i - 1024 * w0
                            nc.vector.tensor_tensor(
                                out=ptiles[w0][:, off : off + 128],
                                in0=ptiles[w0][:, off : off + 128],
                                in1=maskd_t[:, :], op=MULT,
                            )
                        jlo = i // 4 if mode == "causal" else 0
                        for j in range(jlo, 4):
                            last = (
                                (i == 4 * j + 3) if mode == "causal" else (i == NT - 1)
                            )
                            nc.tensor.matmul(
                                pv[j][:, :],
                                Vp[:, i, h, :],
                                ptiles[j // 2][:, (j % 2) * 512 : (j % 2) * 512 + 512],
                                start=(i == 0), stop=last,
                            )
                    # drains: unscaled attn rows + sums row
                    for j in range(4):
                        nc.vector.tensor_copy(
                            out=attnT[hp : hp + 64, P, j * 512 : (j + 1) * 512],
                            in_=pv[j][0:64, :],
                        )
                        nc.vector.tensor_copy(
                            out=sums_st[32 * h : 32 * h + 1, j * 512 : (j + 1) * 512],
                            in_=pv[j][64:65, :],
                        )
                    # softmax denominator: bounce through DRAM to broadcast,
                    # then scale the unscaled PV rows in place
                    nc.sync.dma_start(
                        out=scratch_sums[h : h + 1, :],
                        in_=sums_st[32 * h : 32 * h + 1, :],
                    )
                    bc_ap = bass.AP(
                        tensor=scratch_sums.ap().tensor, offset=h * T,
                        ap=[[0, 64], [1, T]],
                    )
                    nc.sync.dma_start(out=recip_bc[hp : hp + 64, P, :], in_=bc_ap)
                    nc.vector.reciprocal(
                        out=recip_bc[hp : hp + 64, P, :],
                        in_=recip_bc[hp : hp + 64, P, :],
                    )
                    nc.vector.tensor_tensor(
                        out=attnT[hp : hp + 64, P, :],
                        in0=attnT[hp : hp + 64, P, :],
                        in1=recip_bc[hp : hp + 64, P, :], op=MULT,
                    )

              # ---- Phase G: output projection (partial; host reduces)
              with (
                  tc.tile_pool(name="pr_ps", bufs=6, space="PSUM") as prps,
                  tc.tile_pool(name="out_pool", bufs=3) as outp,
              ):
                  for tt in range(NT):
                      o_t = outp.tile([128, C], F32, tag="o")
                      for n in range(2):
                          ps = prps.tile([128, 512], F32, tag="pr", name=f"prps_{tt}_{n}")
                          for P in range(2):
                              nc.tensor.matmul(
                                  ps[:, :],
                                  attnT[:, P, tt * 128 : (tt + 1) * 128],
                                  w_p[:, P, n * 512 : (n + 1) * 512],
                                  start=(P == 0), stop=(P == 1),
                              )
                          drain_eng = nc.vector if n == 0 else nc.scalar
                          if n == 0:
                              nc.vector.tensor_copy(
                                  out=o_t[:, 0:512], in_=ps[:, :]
                              )
                          else:
                              nc.scalar.copy(
                                  out=o_t[:, 512:1024], in_=ps[:, :]
                              )
                      nc.sync.dma_start(
                          out=y_out[tt * 128 : (tt + 1) * 128, :], in_=o_t[:, :]
                      )

    _split_waits(nc, limit=1)
    return nc


def kernel(x, mask, ln_scale, ln_bias, qkv_w, qkv_b, proj_w, proj_b):
    x = np.ascontiguousarray(np.asarray(x), dtype=np.float32)
    mask2 = np.asarray(mask).reshape(T, T)
    ln_scale = np.asarray(ln_scale, dtype=np.float32)
    ln_bias = np.asarray(ln_bias, dtype=np.float32)
    qkv_w = np.asarray(qkv_w, dtype=np.float32)
    qkv_b = np.asarray(qkv_b, dtype=np.float32)
    proj_w = np.asarray(proj_w, dtype=np.float32)
    proj_b = np.asarray(proj_b, dtype=np.float32)

    # fold LayerNorm affine into the qkv projection (exact host-side algebra)
    w_eff = (ln_scale[:, None] * qkv_w).astype(np.float32)
    b_eff = (ln_bias @ qkv_w + qkv_b).astype(np.float32)

    if mask2.all():
        mode = "none"
    elif np.array_equal(mask2, np.tril(np.ones((T, T), dtype=mask2.dtype))):
        mode = "causal"
    else:
        mode = "full"

    in_maps = []
    core_ids = list(range(8))
    vbias = bool(np.any(b_eff[2 * C : 3 * C] != 0.0))
    maskt_f = None
    maskd = None
    if mode == "causal":
        # diag strip masks: for k-tile residue r, the 512-wide diagonal chunk
        # pattern [128, 512]: maskT slice rows [128r,128r+128) cols [0,512)
        maskd = np.ascontiguousarray(mask2[0:128, 0:128].T.astype(np.float32))
    elif mode == "full":
        maskt_f = np.ascontiguousarray(mask2.T.astype(np.float32))

    for core in core_ids:
        b = core // 4
        hs = 4 * (core % 4)
        cols_q = slice(hs * D, hs * D + CL)
        cols_k = slice(C + hs * D, C + hs * D + CL)
        cols_v = slice(2 * C + hs * D, 2 * C + hs * D + CL)
        wl = np.concatenate(
            [w_eff[:, cols_q], w_eff[:, cols_k], w_eff[:, cols_v]], axis=1
        )
        bl = np.concatenate([b_eff[cols_q], b_eff[cols_k], b_eff[cols_v]])
        im = {
            "x_b": np.ascontiguousarray(x[b]),
            "wqkv": np.ascontiguousarray(wl),
            "bqkv": np.ascontiguousarray(bl),
            "wp": np.ascontiguousarray(proj_w[hs * D : hs * D + CL, :]),
        }
        if mode == "causal":
            im["maskd"] = maskd
        elif mode == "full":
            im["maskt"] = maskt_f
        in_maps.append(im)

    nc = _build(mode, vbias)
    trace = bool(int(os.environ.get("KERNEL_TRACE", "0")))
    res = run_bass_kernel_spmd(nc, in_maps, core_ids=core_ids, trace=trace)
    global LAST_RESULT
    LAST_RESULT = res

    out = np.zeros((B, T, C), dtype=np.float32)
    for core in core_ids:
        out[core // 4] += res.results[core]["y_part"]
    out += proj_b[None, None, :]
    return out


if __name__ == "__main__":
    rng = np.random.default_rng(0)
    x = rng.standard_normal((B, T, C), dtype=np.float32)
    mask = np.tril(np.ones((T, T), dtype=bool))[None, None]
    ln_scale = np.ones(C, np.float32)
    ln_bias = np.zeros(C, np.float32)
    lim = float(np.sqrt(6.0 / (C + 3 * C)))
    qkv_w = rng.uniform(-lim, lim, (C, 3 * C)).astype(np.float32)
    qkv_b = np.zeros(3 * C, np.float32)
    limp = float(np.sqrt(6.0 / (C + C)))
    proj_w = rng.uniform(-limp, limp, (C, C)).astype(np.float32)
    proj_b = np.zeros(C, np.float32)
    out = kernel(x, mask, ln_scale, ln_bias, qkv_w, qkv_b, proj_w, proj_b)
    print("out", out.shape, out.dtype, np.abs(out).max())



# revision 68
# speedup vs baseline: 1.5212x; 1.5212x over previous
"""Trainium2 Bass kernel for nn_Attention (pre-LN causal attention block).

Reference computation (B=2, T=2048, C=1024, H=16, D=64, fp32):
    xn = LayerNorm(x)                       (eps=1e-6)
    qkv = xn @ qkv_w + qkv_b;  q,k,v = split(qkv)
    scores = (q @ k^T) / sqrt(D), causal-masked, softmax
    out = (softmax @ v) reshaped @ proj_w + proj_b

Sharding (8 cores): data-parallel over B (cores 0-3 <- batch 0, 4-7 <- batch 1)
x tensor-parallel over heads (4 heads/core: qkv_w column-sharded, proj_w
row-sharded).  Each core emits a partial projection output; the host sums the
4 partials per batch and adds proj_b (the "all-reduce after proj" done
host-side).

Device kernel design notes (v3):
  - All matmuls in bf16 (f32 psum accumulation); bf16 runs at full PE rate at
    ANY free size (fp32r needs >=256), enabling exact causal granularity:
    scores^T tiles for k-tile i cover q in [128i, T) -> 17408 rows/head.
  - Scores are computed TRANSPOSED (s^T[k,q] = K tile @ Q^T) so the PV matmul
    consumes softmax tiles directly: pv[j] += V'[i]^T @ p_i with V' augmented
    by a ones column (row 64 of the psum accumulates sum(p) for free).
    PV output [65, q] rows 0..63 are attn^T -- exactly the proj lhsT layout.
  - Matmul instruction count is kept low (~640): each InstMatmult costs
    ~125ns of PE sequencer decode (Ldweights+Matmult), which rate-limits
    designs with many small matmuls regardless of engine time.
  - Softmax denominators: psum row 64 -> DRAM bounce -> partition-broadcast
    load -> reciprocal -> in-place scale of attn^T, pipelined per 512-column
    j-chunk (chain latency hidden behind later k-tiles / heads).
  - LayerNorm applied as per-partition scale/bias (tensor_scalar) producing
    bf16 xn; one 2-byte DMA-xbar transpose per token tile gives xn^T.  x is
    staged bf16 (host cast, halves x DMA); y partial is bf16 (host f32 sum).
  - Engine balance: exp stream owns Act (QK psum drains go to Act only
    before exps start); LN-apply/V/attn drains split Pool/DVE; all DMA on
    the SP queue ordered so no dispatch-wait blocks a later-needed transfer.
  - Validated numerics (numpy bf16 simulation of this cast structure):
    rel err ~4e-3 vs tolerance 2e-2.
"""

import os

import numpy as np

import concourse.bass as bass
import concourse.tile as tile
from concourse import mybir
from concourse.bass_utils import run_bass_kernel_spmd

LAST_RESULT = None
F32 = mybir.dt.float32
F32R = mybir.dt.float32r
BF16 = mybir.dt.bfloat16

B, T, C = 2, 2048, 1024
H, D = 16, 64
HL = 4            # heads per core
CL = HL * D       # local c-dim (256)
NT = T // 128     # 16 token tiles
NCC = C // 128    # 8 contraction chunks
LN_EPS = 1e-6
SCORE_SCALE = 0.125  # 1/sqrt(D)/TEMP


def _split_waits(nc, limit=1):
    """This container's walrus rejects instructions with >1 sem wait ("Too many
    sync wait commands").  Move excess waits onto same-engine NOPs inserted
    just before the instruction (equivalent under per-engine program order)."""
    n = 0
    for f in nc.m.functions:
        for b in f.blocks:
            insts = b.instructions
            if not any(
                i.sync_info is not None and len(i.sync_info.on_wait) > limit
                for i in insts
            ):
                continue
            new = []
            for inst in insts:
                si = inst.sync_info
                if si is not None and len(si.on_wait) > limit:
                    waits = list(si.on_wait)
                    excess, keep = waits[:-limit], waits[-limit:]
                    for j in range(0, len(excess), limit):
                        n += 1
                        nop = mybir.InstNoOp(name=f"I-wsplit-{n}", ins=[], outs=[])
                        nop.engine = inst.engine
                        nop.sync_info = mybir.SyncInfo(
                            on_wait=excess[j : j + limit], on_update=[]
                        )
                        new.append(nop)
                    inst.sync_info = mybir.SyncInfo(
                        on_wait=keep, on_update=list(si.on_update)
                    )
                new.append(inst)
            b.instructions = new
    return n


def _build(mode, vbias):
    """mode: 'causal' (tile-skip + diag mask), 'none' (no mask), 'full'
    (arbitrary mask, maskT input).  vbias: apply per-column v bias."""
    causal = mode == "causal"
    nc = bass.Bass(name="attnblk")
    x_in = nc.declare_dram_parameter("x_b", [T, C], BF16, isOutput=False)
    wqkv = nc.declare_dram_parameter("wqkv", [C, 3 * CL], BF16, isOutput=False)
    bqkv = nc.declare_dram_parameter("bqkv", [3 * CL], F32, isOutput=False)
    wp = nc.declare_dram_parameter("wp", [CL, C], BF16, isOutput=False)
    if mode == "causal":
        maskd = nc.declare_dram_parameter("maskd", [128, 128], BF16, isOutput=False)
    elif mode == "full":
        maskt = nc.declare_dram_parameter("maskt", [T, T], BF16, isOutput=False)
    y_out = nc.declare_dram_parameter("y_part", [T, C], BF16, isOutput=True)

    Exp = mybir.ActivationFunctionType.Exp
    Sqrt = mybir.ActivationFunctionType.Sqrt
    Ident = mybir.ActivationFunctionType.Identity
    MULT = mybir.AluOpType.mult
    ADD = mybir.AluOpType.add
    SUBTRACT = mybir.AluOpType.subtract

    with tile.TileContext(nc) as tc:
        with (
            tc.tile_pool(name="persist", bufs=1) as pp,
            tc.tile_pool(name="small", bufs=1) as pco,
        ):
            # ---- persistent sbuf tensors
            xnT = pp.tile([128, NCC, T], BF16, tag="xnT")
            qT = pp.tile([128, 2, T], BF16, tag="qT")      # pair-stacked Q^T
            kT = pp.tile([128, 2, T], BF16, tag="kT")
            Vp = pp.tile([128, NT, HL, D + 1], BF16, tag="Vp")  # V' + ones col
            w_qkv = pp.tile([128, NCC, 3 * CL], BF16, tag="wqkv")
            w_p = pp.tile([128, 2, C], BF16, tag="w_p")
            attnT = pp.tile([128, 2, T], BF16, tag="attnT")
            eps_t = pco.tile([128, 1], F32, tag="eps")
            bq_t = pco.tile([128, 2, 1], F32, tag="bq")
            bk_t = pco.tile([128, 2, 1], F32, tag="bk")
            ones_bc = pco.tile([65, D], BF16, tag="ones_bc")
            if mode == "causal":
                maskd_t = pco.tile([128, 128], BF16, tag="maskd")

            nc.vector.memset(eps_t, LN_EPS)
            nc.vector.memset(ones_bc[:, :], 1.0)
            # ones columns for the sums row; V drains overwrite cols 0..D-1
            nc.vector.memset(Vp[:, :, :, :], 1.0)

            with (
                tc.tile_pool(name="sc_ps", bufs=3, space="PSUM") as scp,
                tc.tile_pool(name="v_ps", bufs=1, space="PSUM") as vps,
                tc.tile_pool(name="pv_ps", bufs=4, space="PSUM") as pvp,
                tc.tile_pool(name="x_pool", bufs=16) as xp,
                tc.tile_pool(name="ln_pool", bufs=3) as lnp,
                tc.tile_pool(name="xn_pool", bufs=3) as xnp,
                tc.tile_pool(name="p_pool", bufs=2 if causal else 4) as ppool,
                tc.tile_pool(name="rc_pool", bufs=3) as rcp,
                tc.tile_pool(name="m_pool", bufs=2) as mpool,
                tc.tile_pool(name="out_pool", bufs=3) as outp,
            ):
                x_tiles = {}
                def load_x(tt):
                    x_t = xp.tile([128, C], BF16, tag="x", name=f"x_{tt}")
                    nc.sync.dma_start(
                        out=x_t, in_=x_in[tt * 128 : (tt + 1) * 128, :]
                    )
                    x_tiles[tt] = x_t

                for tt in range(4):
                    load_x(tt)
                nc.sync.dma_start(
                    out=w_qkv, in_=wqkv.ap().rearrange("(cc p) n -> p cc n", p=128)
                )
                for tt in range(4, NT):
                    load_x(tt)
                nc.sync.dma_start(
                    out=w_p, in_=wp.ap().rearrange("(P p) n -> p P n", p=128)
                )
                nc.sync.dma_start(
                    out=bq_t, in_=bqkv.ap()[0:CL].rearrange("(P p) -> p P", p=128)
                )
                nc.sync.dma_start(
                    out=bk_t,
                    in_=bqkv.ap()[CL : 2 * CL].rearrange("(P p) -> p P", p=128),
                )
                if mode == "causal":
                    nc.sync.dma_start(out=maskd_t, in_=maskd[:, :])
                if vbias:
                    bv_t = pco.tile([128, CL], F32, tag="bv")
                    bv_ap = bass.AP(
                        tensor=bqkv.ap().tensor, offset=2 * CL,
                        ap=[[0, 128], [1, CL]],
                    )
                    nc.sync.dma_start(out=bv_t, in_=bv_ap)

                # ---- Phase A: LN -> bf16 xn -> DMA-xbar transpose.
                # Split into two stages emitted with a one-tile lag: the
                # DVE->Act->DVE->Act sem round-trips then overlap the
                # neighboring tile's engine work instead of blocking the
                # in-order engine queues (which would pace the pipeline at
                # the full chain latency, ~2.3us/tile).
                a_state = {}
                def phase_a_stats(tt):
                    x_t = x_tiles.pop(tt)
                    stats = lnp.tile([128, 2, 6], F32, tag="stats")
                    xg = x_t[:, :].rearrange("p (g d) -> p g d", g=2)
                    for g in range(2):
                        nc.vector.bn_stats(out=stats[:, g, :], in_=xg[:, g, :])
                    mv = lnp.tile([128, 2], F32, tag="mv")
                    nc.vector.bn_aggr(out=mv[:, :], in_=stats[:, :, :])
                    rstd = lnp.tile([128, 1], F32, tag="rstd")
                    nc.scalar.activation(
                        out=rstd[:, :], in_=mv[:, 1:2], func=Sqrt,
                        bias=eps_t[:, :], scale=1.0,
                    )
                    a_state[tt] = (x_t, mv, rstd)

                def phase_a_fin(tt):
                    if tt not in a_state:
                        return
                    x_t, mv, rstd = a_state.pop(tt)
                    nc.vector.reciprocal(out=rstd[:, :], in_=rstd[:, :])
                    nmr = lnp.tile([128, 1], F32, tag="nmr")
                    nc.vector.tensor_scalar(
                        nmr[:, :], mv[:, 0:1], rstd[:, :], -1.0,
                        op0=MULT, op1=MULT,
                    )
                    # xn split Act/Pool (sbuf-only op; Pool cannot touch
                    # PSUM so it gets the sbuf work)
                    xn_t = xnp.tile([128, C], BF16, tag="xn")
                    if tt % 2 == 0:
                        nc.scalar.activation(
                            out=xn_t[:, :], in_=x_t[:, :], func=Ident,
                            bias=nmr[:, :], scale=rstd[:, :],
                        )
                    else:
                        nc.gpsimd.tensor_scalar(
                            xn_t[:, :], x_t[:, :], rstd[:, :], nmr[:, :],
                            op0=MULT, op1=ADD,
                        )
                    nc.sync.dma_start(
                        out=xnT[:, :, tt * 128 : (tt + 1) * 128],
                        in_=xn_t[:, :], transpose=True,
                    )

                # ---- Phase B: Q/K projections per 512-token group.
                # P=0 (heads 0,1) is emitted with its group so the exp stream
                # can start right after phase A; P=1 is deferred into head 1's
                # i-loop as PE filler (its drains go to Pool/DVE since Act is
                # then busy with exps).
                def emit_qk(g, which, P, on_act):
                    dest, bias_t = (qT, bq_t) if which == 0 else (kT, bk_t)
                    wcol = which * CL + P * 128
                    ps = scp.tile(
                        [128, 512], F32, tag="sc",
                        name=f"qkps_{g}_{which}_{P}",
                    )
                    for cc in range(NCC):
                        nc.tensor.matmul(
                            ps[:, :],
                            w_qkv[:, cc, wcol : wcol + 128],
                            xnT[:, cc, g * 512 : (g + 1) * 512],
                            start=(cc == 0), stop=(cc == NCC - 1),
                        )
                    if on_act:
                        nc.scalar.activation(
                            out=dest[:, P, g * 512 : (g + 1) * 512],
                            in_=ps[:, :], func=Ident,
                            bias=bias_t[:, P, :], scale=1.0,
                        )
                    else:
                        nc.vector.tensor_scalar(
                            dest[:, P, g * 512 : (g + 1) * 512],
                            ps[:, :], bias_t[:, P, :], None, op0=ADD,
                        )

                # (phase B QK emission is interleaved with phase A and early
                # head-0 score chunks below, after the helpers are defined)

                # ---- V projection (interleaved into head 0's i-loop)
                def emit_v(tt):
                    ps = vps.tile([128, 512], F32, tag="v",
                                  name=f"vps_{tt}")[:, 0:CL]
                    for cc in range(NCC):
                        nc.tensor.matmul(
                            ps[:, :],
                            xnT[:, cc, tt * 128 : (tt + 1) * 128],
                            w_qkv[:, cc, 2 * CL : 3 * CL],
                            start=(cc == 0), stop=(cc == NCC - 1),
                        )
                    psh = ps[:, :].rearrange("p (h d) -> p h d", h=HL)
                    if vbias:
                        bvh = bv_t[:, :].rearrange("p (h d) -> p h d", h=HL)
                        nc.vector.tensor_tensor(
                            out=Vp[:, tt, :, 0:D], in0=psh, in1=bvh, op=ADD
                        )
                    else:
                        nc.vector.tensor_copy(out=Vp[:, tt, :, 0:D], in_=psh)

                # ---- Phase D (emitted interleaved): output projection.
                # use_act: route drains to Act only after the exp stream ends
                npr = 0
                def emit_proj(tt, use_act):
                    nonlocal npr
                    o_t = outp.tile([128, C], BF16, tag="o", name=f"o_{tt}")
                    for n in range(2):
                        ps = scp.tile([128, 512], F32, tag="sc",
                                      name=f"prps_{tt}_{n}")
                        for P in range(2):
                            nc.tensor.matmul(
                                ps[:, :],
                                attnT[:, P, tt * 128 : (tt + 1) * 128],
                                w_p[:, P, n * 512 : (n + 1) * 512],
                                start=(P == 0), stop=(P == 1),
                            )
                        osl = o_t[:, n * 512 : (n + 1) * 512]
                        if use_act and n == 0:
                            nc.scalar.copy(out=osl, in_=ps[:, :])
                        else:
                            npr += 1
                            nc.vector.tensor_copy(out=osl, in_=ps[:, :])
                    nc.sync.dma_start(
                        out=y_out[tt * 128 : (tt + 1) * 128, :], in_=o_t[:, :]
                    )

                # ---- Phase C: attention, one flat software-pipelined slot
                # stream across all heads.  PV consumption lags scores/exp
                # production by a FULL HEAD (LAGM=16 slots) in causal mode:
                # every PV's exp dependency is then a whole head old, so PE
                # never blocks on the exp chain, and PE's surplus work (V,
                # QK-P1, proj) fills the Act-lag inside each score phase.
                LAGM = NT if causal else 2
                pv_tiles, p_tiles = {}, {}
                next_c = {}
                pending_bcast = []

                def flush_bcast():
                    """PE-side broadcast of 1/sums across partitions, emitted
                    one slot late so the DVE reciprocal has time to land."""
                    while pending_bcast:
                        h, j, rc_row = pending_bcast.pop(0)
                        P, hp = h // 2, (h % 2) * 64
                        sl = slice(512 * j, 512 * j + 512)
                        rc_ps = vps.tile([128, 512], F32, tag="v",
                                         name=f"rcps_{h}_{j}")
                        nc.tensor.matmul(
                            rc_ps[hp : hp + 64, :],
                            ones_bc[64:65, 0:64],
                            rc_row[64:65, :],
                            start=True, stop=True,
                        )
                        nc.vector.tensor_tensor(
                            out=attnT[hp : hp + 64, P, sl],
                            in0=attnT[hp : hp + 64, P, sl],
                            in1=rc_ps[hp : hp + 64, :], op=MULT,
                        )

                def emit_score_chunk(h, i):
                    """Emit ONE 512-col score chunk for (h, i).  Returns False
                    when tile i is fully emitted.  Lazily allocates the p tile
                    (exact causal width) and the head's pv psum tiles."""
                    P, hp = h // 2, (h % 2) * 64
                    base = 128 * i if causal else 0
                    width = T - base
                    c0 = next_c.get((h, i), 0)
                    if c0 >= width:
                        return False
                    if c0 == 0:
                        if causal:
                            p_t = ppool.tile([128, width], BF16, tag=f"p{i}",
                                             name=f"p_{h}_{i}")
                        else:
                            p_t = ppool.tile([128, T], BF16, tag="p",
                                             name=f"p_{h}_{i}")
                        p_tiles[(h, i)] = p_t
                    p_t = p_tiles[(h, i)]
                    w = min(512, width - c0)
                    ps = scp.tile([128, 512], F32, tag="sc",
                                  name=f"scps_{h}_{i}_{c0}")
                    nc.tensor.matmul(
                        ps[:, 0:w],
                        kT[hp : hp + D, P, i * 128 : (i + 1) * 128],
                        qT[hp : hp + D, P, base + c0 : base + c0 + w],
                        start=True, stop=True,
                    )
                    nc.scalar.activation(
                        out=p_t[:, c0 : c0 + w], in_=ps[:, 0:w],
                        func=Exp, scale=SCORE_SCALE,
                    )
                    if mode == "full":
                        m_t = mpool.tile([128, 512], BF16, tag="m")
                        nc.sync.dma_start(
                            out=m_t[:, 0:w],
                            in_=maskt[
                                i * 128 : (i + 1) * 128,
                                base + c0 : base + c0 + w,
                            ],
                        )
                        nc.vector.tensor_tensor(
                            out=p_t[:, c0 : c0 + w],
                            in0=p_t[:, c0 : c0 + w],
                            in1=m_t[:, 0:w], op=MULT,
                        )
                    if causal and c0 == 0:
                        nc.gpsimd.tensor_tensor(
                            out=p_t[:, 0:128], in0=p_t[:, 0:128],
                            in1=maskd_t[:, :], op=MULT,
                        )
                    next_c[(h, i)] = c0 + w
                    return True

                def emit_scores_all(h, i):
                    while emit_score_chunk(h, i):
                        pass

                def emit_pv(h, i):
                    P, hp = h // 2, (h % 2) * 64
                    if i == 0:
                        pv_tiles[h] = [
                            pvp.tile([65, 512], F32, tag="pv",
                                     name=f"pv_{h}_{j}")
                            for j in range(4)
                        ]
                    pv = pv_tiles[h]
                    base = 128 * i if causal else 0
                    p_t = p_tiles.pop((h, i))
                    jlo = i // 4 if causal else 0
                    for j in range(jlo, 4):
                        off = 512 * j - base   # local col in p_t
                        o0 = max(0, off)
                        skip = o0 - off        # masked lead columns
                        last = (i == 4 * j + 3) if causal else (i == NT - 1)
                        nc.tensor.matmul(
                            pv[j][:, skip : 512],
                            Vp[:, i, h, :],
                            p_t[:, o0 : off + 512],
                            start=(i == 0), stop=last,
                        )
                        if last:
                            # drain: attn^T rows to sbuf; 1/sums row via DVE
                            # (read straight from the psum sums row); the
                            # cross-partition broadcast happens on PE one
                            # slot later (flush_bcast)
                            sl = slice(512 * j, 512 * j + 512)
                            if j % 2 == 0:
                                nc.scalar.copy(
                                    out=attnT[hp : hp + 64, P, sl],
                                    in_=pv[j][0:64, :],
                                )
                            else:
                                nc.vector.tensor_copy(
                                    out=attnT[hp : hp + 64, P, sl],
                                    in_=pv[j][0:64, :],
                                )
                            rc_row = rcp.tile([65, 512], BF16, tag="rc",
                                              name=f"rc_{h}_{j}")
                            with nc.allow_low_precision(
                                reason="f32r out is bitwise f32"
                            ):
                                nc.vector.reciprocal(
                                    out=rc_row[64:65, :], in_=pv[j][64:65, :]
                                )
                            pending_bcast.append((h, j, rc_row))

                # ---- Phases A+B (+ early head-0 score chunks in causal mode
                # so the Act exp stream starts as soon as QK-P0(g0) lands)
                for g in range(4):
                    for tt in range(4 * g, 4 * g + 4):
                        phase_a_stats(tt)
                        phase_a_fin(tt - 1)
                    phase_a_fin(4 * g + 3)
                    emit_qk(g, 0, 0, False)
                    emit_qk(g, 1, 0, False)
                    if causal:
                        cmax = 512 * (g + 1) if g < 3 else T
                        for i in range(4 * g + 4):
                            base = 128 * i
                            while True:
                                c0 = next_c.get((0, i), 0)
                                w = min(512, T - base - c0)
                                if w <= 0 or base + c0 + w > cmax:
                                    break
                                emit_score_chunk(0, i)

                # ---- main slot stream.  Causal: the whole schedule runs one
                # head-phase early (h0's scores live in phase B), so slots
                # 0-15 carry S(h1)+V, 16-31 S(h2)+PV(h0), 32-47 S(h3)+PV(h1),
                # 48-63 PV(h2), 64-79 PV(h3)+proj.
                if causal:
                    NS, SOFF, PVOFF = (HL - 1) * NT, 1, NT
                else:
                    NS, SOFF, PVOFF = HL * NT, 0, LAGM
                for g in range(HL * NT + PVOFF):
                    flush_bcast()
                    if g < NS:
                        h, i = (g // NT) + SOFF, g % NT
                        emit_scores_all(h, i)
                    if causal:
                        if g < NT:
                            emit_v(g)
                            if g in (1, 3, 5, 7):
                                emit_qk((g - 1) // 2, 0, 1, False)  # q-P1
                            elif g == 9:
                                emit_qk(0, 1, 1, False)             # k-P1 g0
                        elif g in (17, 21, 25):
                            emit_qk((g - 13) // 4, 1, 1, False)     # k-P1 g1-3
                    elif g < NS:
                        if h == 0:
                            emit_v(i)
                        elif h == 1 and i in (1, 3, 5, 7):
                            emit_qk((i - 1) // 2, 0, 1, False)
                        elif h == 1 and i == 9:
                            emit_qk(0, 1, 1, False)
                        elif h == 2 and i in (1, 5, 9):
                            emit_qk(i // 4 + 1, 1, 1, False)
                    if g >= PVOFF:
                        h2, i2 = divmod(g - PVOFF, NT)
                        emit_pv(h2, i2)
                    # proj groups interleave into the PV tail once their
                    # attn^T j-chunks are fully scaled
                    if causal and g in (70, 74, 78):
                        j = (g - 70) // 4
                        for tt in range(4 * j, 4 * j + 4):
                            emit_proj(tt, True)
                flush_bcast()

                for tt in range(12 if causal else 0, NT):
                    emit_proj(tt, True)

    _split_waits(nc, limit=1)
    return nc


def kernel(x, mask, ln_scale, ln_bias, qkv_w, qkv_b, proj_w, proj_b):
    import ml_dtypes

    bf = ml_dtypes.bfloat16
    x = np.ascontiguousarray(np.asarray(x), dtype=np.float32)
    mask2 = np.asarray(mask).reshape(T, T)
    ln_scale = np.asarray(ln_scale, dtype=np.float32)
    ln_bias = np.asarray(ln_bias, dtype=np.float32)
    qkv_w = np.asarray(qkv_w, dtype=np.float32)
    qkv_b = np.asarray(qkv_b, dtype=np.float32)
    proj_w = np.asarray(proj_w, dtype=np.float32)
    proj_b = np.asarray(proj_b, dtype=np.float32)

    # fold LayerNorm affine into the qkv projection (exact host-side algebra)
    w_eff = (ln_scale[:, None] * qkv_w).astype(np.float32)
    b_eff = (ln_bias @ qkv_w + qkv_b).astype(np.float32)

    if mask2.all():
        mode = "none"
    elif np.array_equal(mask2, np.tril(np.ones((T, T), dtype=mask2.dtype))):
        mode = "causal"
    else:
        mode = "full"

    in_maps = []
    core_ids = list(range(8))
    vbias = bool(np.any(b_eff[2 * C : 3 * C] != 0.0))
    maskt_f = None
    maskd_m = None
    if mode == "causal":
        # diag strip mask: maskT[k, q] for the 128x128 diagonal block
        maskd_m = np.ascontiguousarray(mask2[0:128, 0:128].T.astype(bf))
    elif mode == "full":
        maskt_f = np.ascontiguousarray(mask2.T.astype(bf))

    for core in core_ids:
        b = core // 4
        hs = 4 * (core % 4)
        cols_q = slice(hs * D, hs * D + CL)
        cols_k = slice(C + hs * D, C + hs * D + CL)
        cols_v = slice(2 * C + hs * D, 2 * C + hs * D + CL)
        wl = np.concatenate(
            [w_eff[:, cols_q], w_eff[:, cols_k], w_eff[:, cols_v]], axis=1
        )
        bl = np.concatenate([b_eff[cols_q], b_eff[cols_k], b_eff[cols_v]])
        im = {
            "x_b": np.ascontiguousarray(x[b].astype(bf)),
            "wqkv": np.ascontiguousarray(wl.astype(bf)),
            "bqkv": np.ascontiguousarray(bl),
            "wp": np.ascontiguousarray(
                proj_w[hs * D : hs * D + CL, :].astype(bf)
            ),
        }
        if mode == "causal":
            im["maskd"] = maskd_m
        elif mode == "full":
            im["maskt"] = maskt_f
        in_maps.append(im)

    nc = _build(mode, vbias)
    trace = bool(int(os.environ.get("KERNEL_TRACE", "0")))
    res = run_bass_kernel_spmd(nc, in_maps, core_ids=core_ids, trace=trace)
    global LAST_RESULT
    LAST_RESULT = res

    out = np.zeros((B, T, C), dtype=np.float32)
    for core in core_ids:
        out[core // 4] += res.results[core]["y_part"].astype(np.float32)
    out += proj_b[None, None, :]
    return out


if __name__ == "__main__":
    rng = np.random.default_rng(0)
    x = rng.standard_normal((B, T, C), dtype=np.float32)
    mask = np.tril(np.ones((T, T), dtype=bool))[None, None]
    ln_scale = np.ones(C, np.float32)
    ln_bias = np.zeros(C, np.float32)
    lim = float(np.sqrt(6.0 / (C + 3 * C)))
    qkv_w = rng.uniform(-lim, lim, (C, 3 * C)).astype(np.float32)
    qkv_b = np.zeros(3 * C, np.float32)
    limp = float(np.sqrt(6.0 / (C + C)))
    proj_w = rng.uniform(-limp, limp, (C, C)).astype(np.float32)
    proj_b = np.zeros(C, np.float32)
    out = kernel(x, mask, ln_scale, ln_bias, qkv_w, qkv_b, proj_w, proj_b)
    print("out", out.shape, out.dtype, np.abs(out).max())


# revision 70
# speedup vs baseline: 1.5778x; 1.0372x over previous
"""Trainium2 Bass kernel for nn_Attention (pre-LN causal attention block).

Reference computation (B=2, T=2048, C=1024, H=16, D=64, fp32):
    xn = LayerNorm(x)                       (eps=1e-6)
    qkv = xn @ qkv_w + qkv_b;  q,k,v = split(qkv)
    scores = (q @ k^T) / sqrt(D), causal-masked, softmax
    out = (softmax @ v) reshaped @ proj_w + proj_b

Sharding (8 cores): data-parallel over B (cores 0-3 <- batch 0, 4-7 <- batch 1)
x tensor-parallel over heads (4 heads/core: qkv_w column-sharded, proj_w
row-sharded).  Each core emits a partial projection output; the host sums the
4 partials per batch and adds proj_b (the "all-reduce after proj" done
host-side).

Device kernel design notes (measured 174us/core cost-model vs 275us baseline):
  - All matmuls in bf16 (f32 psum accumulation); bf16 runs at full PE rate at
    ANY free size (fp32r needs >=256), enabling exact causal granularity:
    scores^T tiles for k-tile i cover q in [128i, T) -> 17408 rows/head.
  - Scores are computed TRANSPOSED (s^T[k,q] = K tile @ Q^T) so the PV matmul
    consumes softmax tiles directly: pv[j] += V'[i]^T @ p_i with V' augmented
    by a ones column (row 64 of the psum accumulates sum(p) for free).
    PV output [65, q] rows 0..63 are attn^T -- exactly the proj lhsT layout.
  - Matmul instruction count is kept low (~660): each InstMatmult costs
    ~125ns of PE sequencer decode (Ldweights+Matmult), which rate-limits
    designs with many small matmuls regardless of engine time.
  - Softmax denominators: DVE reciprocal reads the psum sums row in place;
    a 1-partition-contraction PE matmul (ones[1,64] x recip_row[1,512], bf16)
    broadcasts it across partitions into psum; DVE scales attn^T in place.
    No DRAM bounce, no cross-engine round trips on the in-order queues.
  - LayerNorm: bn_stats/bn_aggr (DVE) + Sqrt (Act) split into two stages
    emitted one tile apart so cross-engine sem round-trips overlap the
    neighboring tile; apply as per-partition scale/bias on Act/Pool -> bf16
    xn; one 2-byte DMA-xbar transpose per tile gives xn^T.  x is staged bf16
    (host cast, halves x DMA); y partial is bf16 (host f32 sum).
  - Global software pipeline: ALL x loads are issued up front (a DMA dispatch
    that waits holds its queue, so transposes must never precede loads);
    head 0's scores run inside the LN/QKV phase; PV lags scores by a full
    head-phase (exp deps are ~16 slots old when PE consumes them); V, QK-P1
    and the output projection fill PE slack inside later score phases.
  - Engine split obeys "GPSIMD cannot access PSUM": Pool gets sbuf-only work
    (LN apply, diag mask), Act owns the exp stream, DVE takes psum drains.
  - Validated numerics (numpy bf16 simulation of this cast structure):
    rel err ~5e-3 vs tolerance 2e-2; measured on HW: 4.9e-3.
"""

import os

import numpy as np

import concourse.bass as bass
import concourse.tile as tile
from concourse import mybir
from concourse.bass_utils import run_bass_kernel_spmd

LAST_RESULT = None
F32 = mybir.dt.float32
F32R = mybir.dt.float32r
BF16 = mybir.dt.bfloat16

B, T, C = 2, 2048, 1024
H, D = 16, 64
HL = 4            # heads per core
CL = HL * D       # local c-dim (256)
NT = T // 128     # 16 token tiles
NCC = C // 128    # 8 contraction chunks
LN_EPS = 1e-6
SCORE_SCALE = 0.125  # 1/sqrt(D)/TEMP


def _split_waits(nc, limit=1):
    """This container's walrus rejects instructions with >1 sem wait ("Too many
    sync wait commands").  Move excess waits onto same-engine NOPs inserted
    just before the instruction (equivalent under per-engine program order)."""
    n = 0
    for f in nc.m.functions:
        for b in f.blocks:
            insts = b.instructions
            if not any(
                i.sync_info is not None and len(i.sync_info.on_wait) > limit
                for i in insts
            ):
                continue
            new = []
            for inst in insts:
                si = inst.sync_info
                if si is not None and len(si.on_wait) > limit:
                    waits = list(si.on_wait)
                    excess, keep = waits[:-limit], waits[-limit:]
                    for j in range(0, len(excess), limit):
                        n += 1
                        nop = mybir.InstNoOp(name=f"I-wsplit-{n}", ins=[], outs=[])
                        nop.engine = inst.engine
                        nop.sync_info = mybir.SyncInfo(
                            on_wait=excess[j : j + limit], on_update=[]
                        )
                        new.append(nop)
                    inst.sync_info = mybir.SyncInfo(
                        on_wait=keep, on_update=list(si.on_update)
                    )
                new.append(inst)
            b.instructions = new
    return n


def _build(mode, vbias):
    """mode: 'causal' (tile-skip + diag mask), 'none' (no mask), 'full'
    (arbitrary mask, maskT input).  vbias: apply per-column v bias."""
    causal = mode == "causal"
    nc = bass.Bass(name="attnblk")
    x_in = nc.declare_dram_parameter("x_b", [T, C], BF16, isOutput=False)
    wqkv = nc.declare_dram_parameter("wqkv", [C, 3 * CL], BF16, isOutput=False)
    bqkv = nc.declare_dram_parameter("bqkv", [3 * CL], F32, isOutput=False)
    wp = nc.declare_dram_parameter("wp", [CL, C], BF16, isOutput=False)
    if mode == "causal":
        maskd = nc.declare_dram_parameter("maskd", [128, 128], BF16, isOutput=False)
    elif mode == "full":
        maskt = nc.declare_dram_parameter("maskt", [T, T], BF16, isOutput=False)
    y_out = nc.declare_dram_parameter("y_part", [T, C], BF16, isOutput=True)

    Exp = mybir.ActivationFunctionType.Exp
    Sqrt = mybir.ActivationFunctionType.Sqrt
    Ident = mybir.ActivationFunctionType.Identity
    MULT = mybir.AluOpType.mult
    ADD = mybir.AluOpType.add
    SUBTRACT = mybir.AluOpType.subtract

    with tile.TileContext(nc) as tc:
        with (
            tc.tile_pool(name="persist", bufs=1) as pp,
            tc.tile_pool(name="small", bufs=1) as pco,
        ):
            # ---- persistent sbuf tensors
            xnT = pp.tile([128, NCC, T], BF16, tag="xnT")
            qT = pp.tile([128, 2, T], BF16, tag="qT")      # pair-stacked Q^T
            kT = pp.tile([128, 2, T], BF16, tag="kT")
            Vp = pp.tile([128, NT, HL, D + 1], BF16, tag="Vp")  # V' + ones col
            w_qkv = pp.tile([128, NCC, 3 * CL], BF16, tag="wqkv")
            w_p = pp.tile([128, 2, C], BF16, tag="w_p")
            attnT = pp.tile([128, 2, T], BF16, tag="attnT")
            eps_t = pco.tile([128, 1], F32, tag="eps")
            bq_t = pco.tile([128, 2, 1], F32, tag="bq")
            bk_t = pco.tile([128, 2, 1], F32, tag="bk")
            ones_bc = pco.tile([65, D], BF16, tag="ones_bc")
            if mode == "causal":
                maskd_t = pco.tile([128, 128], BF16, tag="maskd")

            nc.vector.memset(eps_t, LN_EPS)
            nc.vector.memset(ones_bc[:, :], 1.0)
            # ones columns for the sums row; V drains overwrite cols 0..D-1
            nc.vector.memset(Vp[:, :, :, :], 1.0)

            with (
                tc.tile_pool(name="sc_ps", bufs=3, space="PSUM") as scp,
                tc.tile_pool(name="v_ps", bufs=1, space="PSUM") as vps,
                tc.tile_pool(name="pv_ps", bufs=4, space="PSUM") as pvp,
                tc.tile_pool(name="x_pool", bufs=16) as xp,
                tc.tile_pool(name="ln_pool", bufs=3) as lnp,
                tc.tile_pool(name="xn_pool", bufs=3) as xnp,
                tc.tile_pool(name="p_pool", bufs=2 if causal else 4) as ppool,
                tc.tile_pool(name="rc_pool", bufs=3) as rcp,
                tc.tile_pool(name="m_pool", bufs=2) as mpool,
                tc.tile_pool(name="out_pool", bufs=3) as outp,
            ):
                x_tiles = {}
                def load_x(tt):
                    x_t = xp.tile([128, C], BF16, tag="x", name=f"x_{tt}")
                    nc.sync.dma_start(
                        out=x_t, in_=x_in[tt * 128 : (tt + 1) * 128, :]
                    )
                    x_tiles[tt] = x_t

                for tt in range(4):
                    load_x(tt)
                nc.sync.dma_start(
                    out=w_qkv, in_=wqkv.ap().rearrange("(cc p) n -> p cc n", p=128)
                )
                for tt in range(4, NT):
                    load_x(tt)
                nc.sync.dma_start(
                    out=w_p, in_=wp.ap().rearrange("(P p) n -> p P n", p=128)
                )
                nc.sync.dma_start(
                    out=bq_t, in_=bqkv.ap()[0:CL].rearrange("(P p) -> p P", p=128)
                )
                nc.sync.dma_start(
                    out=bk_t,
                    in_=bqkv.ap()[CL : 2 * CL].rearrange("(P p) -> p P", p=128),
                )
                if mode == "causal":
                    nc.sync.dma_start(out=maskd_t, in_=maskd[:, :])
                if vbias:
                    bv_t = pco.tile([128, CL], F32, tag="bv")
                    bv_ap = bass.AP(
                        tensor=bqkv.ap().tensor, offset=2 * CL,
                        ap=[[0, 128], [1, CL]],
                    )
                    nc.sync.dma_start(out=bv_t, in_=bv_ap)

                # ---- Phase A: LN -> bf16 xn -> DMA-xbar transpose.
                # Split into two stages emitted with a one-tile lag: the
                # DVE->Act->DVE->Act sem round-trips then overlap the
                # neighboring tile's engine work instead of blocking the
                # in-order engine queues (which would pace the pipeline at
                # the full chain latency, ~2.3us/tile).
                a_state = {}
                def phase_a_stats(tt):
                    x_t = x_tiles.pop(tt)
                    stats = lnp.tile([128, 2, 6], F32, tag="stats")
                    xg = x_t[:, :].rearrange("p (g d) -> p g d", g=2)
                    for g in range(2):
                        nc.vector.bn_stats(out=stats[:, g, :], in_=xg[:, g, :])
                    mv = lnp.tile([128, 2], F32, tag="mv")
                    nc.vector.bn_aggr(out=mv[:, :], in_=stats[:, :, :])
                    rstd = lnp.tile([128, 1], F32, tag="rstd")
                    nc.scalar.activation(
                        out=rstd[:, :], in_=mv[:, 1:2], func=Sqrt,
                        bias=eps_t[:, :], scale=1.0,
                    )
                    a_state[tt] = (x_t, mv, rstd)

                def phase_a_fin(tt):
                    if tt not in a_state:
                        return
                    x_t, mv, rstd = a_state.pop(tt)
                    nc.vector.reciprocal(out=rstd[:, :], in_=rstd[:, :])
                    nmr = lnp.tile([128, 1], F32, tag="nmr")
                    nc.vector.tensor_scalar(
                        nmr[:, :], mv[:, 0:1], rstd[:, :], -1.0,
                        op0=MULT, op1=MULT,
                    )
                    # xn split Act/Pool (sbuf-only op; Pool cannot touch
                    # PSUM so it gets the sbuf work)
                    xn_t = xnp.tile([128, C], BF16, tag="xn")
                    if tt % 2 == 0:
                        nc.scalar.activation(
                            out=xn_t[:, :], in_=x_t[:, :], func=Ident,
                            bias=nmr[:, :], scale=rstd[:, :],
                        )
                    else:
                        nc.gpsimd.tensor_scalar(
                            xn_t[:, :], x_t[:, :], rstd[:, :], nmr[:, :],
                            op0=MULT, op1=ADD,
                        )
                    nc.sync.dma_start(
                        out=xnT[:, :, tt * 128 : (tt + 1) * 128],
                        in_=xn_t[:, :], transpose=True,
                    )

                # ---- Phase B: Q/K projections per 512-token group.
                # P=0 (heads 0,1) is emitted with its group so the exp stream
                # can start right after phase A; P=1 is deferred into head 1's
                # i-loop as PE filler (its drains go to Pool/DVE since Act is
                # then busy with exps).
                def emit_qk(g, which, P, on_act):
                    dest, bias_t = (qT, bq_t) if which == 0 else (kT, bk_t)
                    wcol = which * CL + P * 128
                    ps = scp.tile(
                        [128, 512], F32, tag="sc",
                        name=f"qkps_{g}_{which}_{P}",
                    )
                    for cc in range(NCC):
                        nc.tensor.matmul(
                            ps[:, :],
                            w_qkv[:, cc, wcol : wcol + 128],
                            xnT[:, cc, g * 512 : (g + 1) * 512],
                            start=(cc == 0), stop=(cc == NCC - 1),
                        )
                    if on_act:
                        nc.scalar.activation(
                            out=dest[:, P, g * 512 : (g + 1) * 512],
                            in_=ps[:, :], func=Ident,
                            bias=bias_t[:, P, :], scale=1.0,
                        )
                    else:
                        nc.vector.tensor_scalar(
                            dest[:, P, g * 512 : (g + 1) * 512],
                            ps[:, :], bias_t[:, P, :], None, op0=ADD,
                        )

                # (phase B QK emission is interleaved with phase A and early
                # head-0 score chunks below, after the helpers are defined)

                # ---- V projection (interleaved into head 0's i-loop)
                def emit_v(tt):
                    ps = vps.tile([128, 512], F32, tag="v",
                                  name=f"vps_{tt}")[:, 0:CL]
                    for cc in range(NCC):
                        nc.tensor.matmul(
                            ps[:, :],
                            xnT[:, cc, tt * 128 : (tt + 1) * 128],
                            w_qkv[:, cc, 2 * CL : 3 * CL],
                            start=(cc == 0), stop=(cc == NCC - 1),
                        )
                    psh = ps[:, :].rearrange("p (h d) -> p h d", h=HL)
                    if vbias:
                        bvh = bv_t[:, :].rearrange("p (h d) -> p h d", h=HL)
                        nc.vector.tensor_tensor(
                            out=Vp[:, tt, :, 0:D], in0=psh, in1=bvh, op=ADD
                        )
                    else:
                        nc.vector.tensor_copy(out=Vp[:, tt, :, 0:D], in_=psh)

                # ---- Phase D (emitted interleaved): output projection.
                # use_act: route drains to Act only after the exp stream ends
                npr = 0
                def emit_proj(tt, use_act):
                    nonlocal npr
                    o_t = outp.tile([128, C], BF16, tag="o", name=f"o_{tt}")
                    for n in range(2):
                        ps = scp.tile([128, 512], F32, tag="sc",
                                      name=f"prps_{tt}_{n}")
                        for P in range(2):
                            nc.tensor.matmul(
                                ps[:, :],
                                attnT[:, P, tt * 128 : (tt + 1) * 128],
                                w_p[:, P, n * 512 : (n + 1) * 512],
                                start=(P == 0), stop=(P == 1),
                            )
                        osl = o_t[:, n * 512 : (n + 1) * 512]
                        if use_act and n == 0:
                            nc.scalar.copy(out=osl, in_=ps[:, :])
                        else:
                            npr += 1
                            nc.vector.tensor_copy(out=osl, in_=ps[:, :])
                    nc.sync.dma_start(
                        out=y_out[tt * 128 : (tt + 1) * 128, :], in_=o_t[:, :]
                    )

                # ---- Phase C: attention, one flat software-pipelined slot
                # stream across all heads.  PV consumption lags scores/exp
                # production by a FULL HEAD (LAGM=16 slots) in causal mode:
                # every PV's exp dependency is then a whole head old, so PE
                # never blocks on the exp chain, and PE's surplus work (V,
                # QK-P1, proj) fills the Act-lag inside each score phase.
                LAGM = NT if causal else 2
                pv_tiles, p_tiles = {}, {}
                next_c = {}
                pending_bcast = []

                def flush_bcast():
                    """PE-side broadcast of 1/sums across partitions, emitted
                    one slot late so the DVE reciprocal has time to land."""
                    while pending_bcast:
                        h, j, rc_row = pending_bcast.pop(0)
                        P, hp = h // 2, (h % 2) * 64
                        sl = slice(512 * j, 512 * j + 512)
                        rc_ps = vps.tile([128, 512], F32, tag="v",
                                         name=f"rcps_{h}_{j}")
                        nc.tensor.matmul(
                            rc_ps[hp : hp + 64, :],
                            ones_bc[64:65, 0:64],
                            rc_row[64:65, :],
                            start=True, stop=True,
                        )
                        nc.vector.tensor_tensor(
                            out=attnT[hp : hp + 64, P, sl],
                            in0=attnT[hp : hp + 64, P, sl],
                            in1=rc_ps[hp : hp + 64, :], op=MULT,
                        )

                def emit_score_chunk(h, i):
                    """Emit ONE 512-col score chunk for (h, i).  Returns False
                    when tile i is fully emitted.  Lazily allocates the p tile
                    (exact causal width) and the head's pv psum tiles."""
                    P, hp = h // 2, (h % 2) * 64
                    base = 128 * i if causal else 0
                    width = T - base
                    c0 = next_c.get((h, i), 0)
                    if c0 >= width:
                        return False
                    if c0 == 0:
                        if causal:
                            p_t = ppool.tile([128, width], BF16, tag=f"p{i}",
                                             name=f"p_{h}_{i}")
                        else:
                            p_t = ppool.tile([128, T], BF16, tag="p",
                                             name=f"p_{h}_{i}")
                        p_tiles[(h, i)] = p_t
                    p_t = p_tiles[(h, i)]
                    w = min(512, width - c0)
                    ps = scp.tile([128, 512], F32, tag="sc",
                                  name=f"scps_{h}_{i}_{c0}")
                    nc.tensor.matmul(
                        ps[:, 0:w],
                        kT[hp : hp + D, P, i * 128 : (i + 1) * 128],
                        qT[hp : hp + D, P, base + c0 : base + c0 + w],
                        start=True, stop=True,
                    )
                    nc.scalar.activation(
                        out=p_t[:, c0 : c0 + w], in_=ps[:, 0:w],
                        func=Exp, scale=SCORE_SCALE,
                    )
                    if mode == "full":
                        m_t = mpool.tile([128, 512], BF16, tag="m")
                        nc.sync.dma_start(
                            out=m_t[:, 0:w],
                            in_=maskt[
                                i * 128 : (i + 1) * 128,
                                base + c0 : base + c0 + w,
                            ],
                        )
                        nc.vector.tensor_tensor(
                            out=p_t[:, c0 : c0 + w],
                            in0=p_t[:, c0 : c0 + w],
                            in1=m_t[:, 0:w], op=MULT,
                        )
                    if causal and c0 == 0:
                        nc.gpsimd.tensor_tensor(
                            out=p_t[:, 0:128], in0=p_t[:, 0:128],
                            in1=maskd_t[:, :], op=MULT,
                        )
                    next_c[(h, i)] = c0 + w
                    return True

                def emit_scores_all(h, i):
                    while emit_score_chunk(h, i):
                        pass

                def emit_pv(h, i):
                    P, hp = h // 2, (h % 2) * 64
                    if i == 0:
                        pv_tiles[h] = [
                            pvp.tile([65, 512], F32, tag="pv",
                                     name=f"pv_{h}_{j}")
                            for j in range(4)
                        ]
                    pv = pv_tiles[h]
                    base = 128 * i if causal else 0
                    p_t = p_tiles.pop((h, i))
                    jlo = i // 4 if causal else 0
                    for j in range(jlo, 4):
                        off = 512 * j - base   # local col in p_t
                        o0 = max(0, off)
                        skip = o0 - off        # masked lead columns
                        last = (i == 4 * j + 3) if causal else (i == NT - 1)
                        nc.tensor.matmul(
                            pv[j][:, skip : 512],
                            Vp[:, i, h, :],
                            p_t[:, o0 : off + 512],
                            start=(i == 0), stop=last,
                        )
                        if last:
                            # drain: attn^T rows to sbuf; 1/sums row via DVE
                            # (read straight from the psum sums row); the
                            # cross-partition broadcast happens on PE one
                            # slot later (flush_bcast)
                            sl = slice(512 * j, 512 * j + 512)
                            nc.vector.tensor_copy(
                                out=attnT[hp : hp + 64, P, sl],
                                in_=pv[j][0:64, :],
                            )
                            rc_row = rcp.tile([65, 512], BF16, tag="rc",
                                              name=f"rc_{h}_{j}")
                            with nc.allow_low_precision(
                                reason="f32r out is bitwise f32"
                            ):
                                nc.vector.reciprocal(
                                    out=rc_row[64:65, :], in_=pv[j][64:65, :]
                                )
                            pending_bcast.append((h, j, rc_row))

                # ---- Phases A+B (+ early head-0 score chunks in causal mode
                # so the Act exp stream starts as soon as QK-P0(g0) lands)
                for g in range(4):
                    for tt in range(4 * g, 4 * g + 4):
                        phase_a_stats(tt)
                        phase_a_fin(tt - 1)
                    phase_a_fin(4 * g + 3)
                    emit_qk(g, 0, 0, False)
                    emit_qk(g, 1, 0, False)
                    if causal:
                        cmax = 512 * (g + 1) if g < 3 else T
                        for i in range(4 * g + 4):
                            base = 128 * i
                            while True:
                                c0 = next_c.get((0, i), 0)
                                w = min(512, T - base - c0)
                                if w <= 0 or base + c0 + w > cmax:
                                    break
                                emit_score_chunk(0, i)

                # ---- main slot stream.  Causal: the whole schedule runs one
                # head-phase early (h0's scores live in phase B), so slots
                # 0-15 carry S(h1)+V, 16-31 S(h2)+PV(h0), 32-47 S(h3)+PV(h1),
                # 48-63 PV(h2), 64-79 PV(h3)+proj.
                if causal:
                    NS, SOFF, PVOFF = (HL - 1) * NT, 1, NT
                else:
                    NS, SOFF, PVOFF = HL * NT, 0, LAGM
                for g in range(HL * NT + PVOFF):
                    flush_bcast()
                    if g < NS:
                        h, i = (g // NT) + SOFF, g % NT
                        emit_scores_all(h, i)
                    if causal:
                        if g < NT:
                            emit_v(g)
                            if g in (1, 3, 5, 7):
                                emit_qk((g - 1) // 2, 0, 1, False)  # q-P1
                            elif g == 9:
                                emit_qk(0, 1, 1, False)             # k-P1 g0
                        elif g in (17, 21, 25):
                            emit_qk((g - 13) // 4, 1, 1, False)     # k-P1 g1-3
                    elif g < NS:
                        if h == 0:
                            emit_v(i)
                        elif h == 1 and i in (1, 3, 5, 7):
                            emit_qk((i - 1) // 2, 0, 1, False)
                        elif h == 1 and i == 9:
                            emit_qk(0, 1, 1, False)
                        elif h == 2 and i in (1, 5, 9):
                            emit_qk(i // 4 + 1, 1, 1, False)
                    if g >= PVOFF:
                        h2, i2 = divmod(g - PVOFF, NT)
                        emit_pv(h2, i2)
                    # proj groups interleave into the PV tail once their
                    # attn^T j-chunks are fully scaled
                    if causal and g in (70, 74, 78):
                        j = (g - 70) // 4
                        for tt in range(4 * j, 4 * j + 4):
                            emit_proj(tt, True)
                flush_bcast()

                for tt in range(12 if causal else 0, NT):
                    emit_proj(tt, True)

    _split_waits(nc, limit=1)
    return nc


def kernel(x, mask, ln_scale, ln_bias, qkv_w, qkv_b, proj_w, proj_b):
    import ml_dtypes

    bf = ml_dtypes.bfloat16
    x = np.ascontiguousarray(np.asarray(x), dtype=np.float32)
    mask2 = np.asarray(mask).reshape(T, T)
    ln_scale = np.asarray(ln_scale, dtype=np.float32)
    ln_bias = np.asarray(ln_bias, dtype=np.float32)
    qkv_w = np.asarray(qkv_w, dtype=np.float32)
    qkv_b = np.asarray(qkv_b, dtype=np.float32)
    proj_w = np.asarray(proj_w, dtype=np.float32)
    proj_b = np.asarray(proj_b, dtype=np.float32)

    # fold LayerNorm affine into the qkv projection (exact host-side algebra)
    w_eff = (ln_scale[:, None] * qkv_w).astype(np.float32)
    b_eff = (ln_bias @ qkv_w + qkv_b).astype(np.float32)

    if mask2.all():
        mode = "none"
    elif np.array_equal(mask2, np.tril(np.ones((T, T), dtype=mask2.dtype))):
        mode = "causal"
    else:
        mode = "full"

    in_maps = []
    core_ids = list(range(8))
    vbias = bool(np.any(b_eff[2 * C : 3 * C] != 0.0))
    maskt_f = None
    maskd_m = None
    if mode == "causal":
        # diag strip mask: maskT[k, q] for the 128x128 diagonal block
        maskd_m = np.ascontiguousarray(mask2[0:128, 0:128].T.astype(bf))
    elif mode == "full":
        maskt_f = np.ascontiguousarray(mask2.T.astype(bf))

    for core in core_ids:
        b = core // 4
        hs = 4 * (core % 4)
        cols_q = slice(hs * D, hs * D + CL)
        cols_k = slice(C + hs * D, C + hs * D + CL)
        cols_v = slice(2 * C + hs * D, 2 * C + hs * D + CL)
        wl = np.concatenate(
            [w_eff[:, cols_q], w_eff[:, cols_k], w_eff[:, cols_v]], axis=1
        )
        bl = np.concatenate([b_eff[cols_q], b_eff[cols_k], b_eff[cols_v]])
        im = {
            "x_b": np.ascontiguousarray(x[b].astype(bf)),
            "wqkv": np.ascontiguousarray(wl.astype(bf)),
            "bqkv": np.ascontiguousarray(bl),
            "wp": np.ascontiguousarray(
                proj_w[hs * D : hs * D + CL, :].astype(bf)
            ),
        }
        if mode == "causal":
            im["maskd"] = maskd_m
        elif mode == "full":
            im["maskt"] = maskt_f
        in_maps.append(im)

    nc = _build(mode, vbias)
    trace = bool(int(os.environ.get("KERNEL_TRACE", "0")))
    res = run_bass_kernel_spmd(nc, in_maps, core_ids=core_ids, trace=trace)
    global LAST_RESULT
    LAST_RESULT = res

    out = np.zeros((B, T, C), dtype=np.float32)
    for core in core_ids:
        out[core // 4] += res.results[core]["y_part"].astype(np.float32)
    out += proj_b[None, None, :]
    return out


if __name__ == "__main__":
    rng = np.random.default_rng(0)
    x = rng.standard_normal((B, T, C), dtype=np.float32)
    mask = np.tril(np.ones((T, T), dtype=bool))[None, None]
    ln_scale = np.ones(C, np.float32)
    ln_bias = np.zeros(C, np.float32)
    lim = float(np.sqrt(6.0 / (C + 3 * C)))
    qkv_w = rng.uniform(-lim, lim, (C, 3 * C)).astype(np.float32)
    qkv_b = np.zeros(3 * C, np.float32)
    limp = float(np.sqrt(6.0 / (C + C)))
    proj_w = rng.uniform(-limp, limp, (C, C)).astype(np.float32)
    proj_b = np.zeros(C, np.float32)
    out = kernel(x, mask, ln_scale, ln_bias, qkv_w, qkv_b, proj_w, proj_b)
    print("out", out.shape, out.dtype, np.abs(out).max())


# revision 74
# speedup vs baseline: 1.6386x; 1.0386x over previous
"""Trainium2 Bass kernel for nn_Attention (pre-LN causal attention block).

Reference computation (B=2, T=2048, C=1024, H=16, D=64, fp32):
    xn = LayerNorm(x)                       (eps=1e-6)
    qkv = xn @ qkv_w + qkv_b;  q,k,v = split(qkv)
    scores = (q @ k^T) / sqrt(D), causal-masked, softmax
    out = (softmax @ v) reshaped @ proj_w + proj_b

Sharding (8 cores): data-parallel over B (cores 0-3 <- batch 0, 4-7 <- batch 1)
x tensor-parallel over heads (4 heads/core: qkv_w column-sharded, proj_w
row-sharded).  Each core emits a partial projection output; the host sums the
4 partials per batch and adds proj_b (the "all-reduce after proj" done
host-side).

Device kernel design notes (measured 174us/core cost-model vs 275us baseline):
  - All matmuls in bf16 (f32 psum accumulation); bf16 runs at full PE rate at
    ANY free size (fp32r needs >=256), enabling exact causal granularity:
    scores^T tiles for k-tile i cover q in [128i, T) -> 17408 rows/head.
  - Scores are computed TRANSPOSED (s^T[k,q] = K tile @ Q^T) so the PV matmul
    consumes softmax tiles directly: pv[j] += V'[i]^T @ p_i with V' augmented
    by a ones column (row 64 of the psum accumulates sum(p) for free).
    PV output [65, q] rows 0..63 are attn^T -- exactly the proj lhsT layout.
  - Matmul instruction count is kept low (~660): each InstMatmult costs
    ~125ns of PE sequencer decode (Ldweights+Matmult), which rate-limits
    designs with many small matmuls regardless of engine time.
  - Softmax denominators: DVE reciprocal reads the psum sums row in place;
    a 1-partition-contraction PE matmul (ones[1,64] x recip_row[1,512], bf16)
    broadcasts it across partitions into psum; DVE scales attn^T in place.
    No DRAM bounce, no cross-engine round trips on the in-order queues.
  - LayerNorm: bn_stats/bn_aggr (DVE) + Sqrt (Act) split into two stages
    emitted one tile apart so cross-engine sem round-trips overlap the
    neighboring tile; apply as per-partition scale/bias on Act/Pool -> bf16
    xn; one 2-byte DMA-xbar transpose per tile gives xn^T.  x is staged bf16
    (host cast, halves x DMA); y partial is bf16 (host f32 sum).
  - Global software pipeline: ALL x loads are issued up front (a DMA dispatch
    that waits holds its queue, so transposes must never precede loads);
    head 0's scores run inside the LN/QKV phase; PV lags scores by a full
    head-phase (exp deps are ~16 slots old when PE consumes them); V, QK-P1
    and the output projection fill PE slack inside later score phases.
  - Engine split obeys "GPSIMD cannot access PSUM": Pool gets sbuf-only work
    (LN apply, diag mask), Act owns the exp stream, DVE takes psum drains.
  - Validated numerics (numpy bf16 simulation of this cast structure):
    rel err ~5e-3 vs tolerance 2e-2; measured on HW: 4.9e-3.
"""

import os

import numpy as np

import concourse.bass as bass
import concourse.tile as tile
from concourse import mybir
from concourse.bass_utils import run_bass_kernel_spmd

LAST_RESULT = None
F32 = mybir.dt.float32
F32R = mybir.dt.float32r
BF16 = mybir.dt.bfloat16

B, T, C = 2, 2048, 1024
H, D = 16, 64
HL = 4            # heads per core
CL = HL * D       # local c-dim (256)
NT = T // 128     # 16 token tiles
NCC = C // 128    # 8 contraction chunks
LN_EPS = 1e-6
SCORE_SCALE = 0.125  # 1/sqrt(D)/TEMP


def _split_waits(nc, limit=1):
    """This container's walrus rejects instructions with >1 sem wait ("Too many
    sync wait commands").  Move excess waits onto same-engine NOPs inserted
    just before the instruction (equivalent under per-engine program order)."""
    n = 0
    for f in nc.m.functions:
        for b in f.blocks:
            insts = b.instructions
            if not any(
                i.sync_info is not None and len(i.sync_info.on_wait) > limit
                for i in insts
            ):
                continue
            new = []
            for inst in insts:
                si = inst.sync_info
                if si is not None and len(si.on_wait) > limit:
                    waits = list(si.on_wait)
                    excess, keep = waits[:-limit], waits[-limit:]
                    for j in range(0, len(excess), limit):
                        n += 1
                        nop = mybir.InstNoOp(name=f"I-wsplit-{n}", ins=[], outs=[])
                        nop.engine = inst.engine
                        nop.sync_info = mybir.SyncInfo(
                            on_wait=excess[j : j + limit], on_update=[]
                        )
                        new.append(nop)
                    inst.sync_info = mybir.SyncInfo(
                        on_wait=keep, on_update=list(si.on_update)
                    )
                new.append(inst)
            b.instructions = new
    return n


def _build(mode, vbias):
    """mode: 'causal' (tile-skip + diag mask), 'none' (no mask), 'full'
    (arbitrary mask, maskT input).  vbias: apply per-column v bias."""
    causal = mode == "causal"
    nc = bass.Bass(name="attnblk")
    x_in = nc.declare_dram_parameter("x_b", [T, C], BF16, isOutput=False)
    wqkv = nc.declare_dram_parameter("wqkv", [C, 3 * CL], BF16, isOutput=False)
    bqkv = nc.declare_dram_parameter("bqkv", [3 * CL], F32, isOutput=False)
    wp = nc.declare_dram_parameter("wp", [CL, C], BF16, isOutput=False)
    if mode == "causal":
        maskd = nc.declare_dram_parameter("maskd", [128, 128], BF16, isOutput=False)
    elif mode == "full":
        maskt = nc.declare_dram_parameter("maskt", [T, T], BF16, isOutput=False)
    y_out = nc.declare_dram_parameter("y_part", [T, C], BF16, isOutput=True)

    Exp = mybir.ActivationFunctionType.Exp
    Sqrt = mybir.ActivationFunctionType.Sqrt
    Ident = mybir.ActivationFunctionType.Identity
    MULT = mybir.AluOpType.mult
    ADD = mybir.AluOpType.add
    SUBTRACT = mybir.AluOpType.subtract

    with tile.TileContext(nc) as tc:
        with (
            tc.tile_pool(name="persist", bufs=1) as pp,
            tc.tile_pool(name="small", bufs=1) as pco,
        ):
            # ---- persistent sbuf tensors
            xnT = pp.tile([128, NCC, T], BF16, tag="xnT")
            qT = pp.tile([128, 2, T], BF16, tag="qT")      # pair-stacked Q^T
            kT = pp.tile([128, 2, T], BF16, tag="kT")
            Vp = pp.tile([128, NT, HL, D + 1], BF16, tag="Vp")  # V' + ones col
            w_qkv = pp.tile([128, NCC, 3 * CL], BF16, tag="wqkv")
            w_p = pp.tile([128, 2, C], BF16, tag="w_p")
            attnT = pp.tile([128, 2, T], BF16, tag="attnT")
            eps_t = pco.tile([128, 1], F32, tag="eps")
            bq_t = pco.tile([128, 2, 1], F32, tag="bq")
            bk_t = pco.tile([128, 2, 1], F32, tag="bk")
            ones_bc = pco.tile([65, D], BF16, tag="ones_bc")
            if mode == "causal":
                maskd_t = pco.tile([128, 128], BF16, tag="maskd")

            nc.vector.memset(eps_t, LN_EPS)
            nc.vector.memset(ones_bc[:, :], 1.0)
            # ones columns for the sums row; V drains overwrite cols 0..D-1
            nc.vector.memset(Vp[:, :, :, :], 1.0)

            with (
                tc.tile_pool(name="sc_ps", bufs=3, space="PSUM") as scp,
                tc.tile_pool(name="v_ps", bufs=1, space="PSUM") as vps,
                tc.tile_pool(name="pv_ps", bufs=4, space="PSUM") as pvp,
                tc.tile_pool(name="x_pool", bufs=16) as xp,
                tc.tile_pool(name="ln_pool", bufs=4) as lnp,
                tc.tile_pool(name="xn_pool", bufs=3) as xnp,
                tc.tile_pool(name="p_pool", bufs=2 if causal else 4) as ppool,
                tc.tile_pool(name="rc_pool", bufs=3) as rcp,
                tc.tile_pool(name="m_pool", bufs=2) as mpool,
                tc.tile_pool(name="out_pool", bufs=3) as outp,
            ):
                x_tiles = {}
                def load_x(tt):
                    x_t = xp.tile([128, C], BF16, tag="x", name=f"x_{tt}")
                    nc.sync.dma_start(
                        out=x_t, in_=x_in[tt * 128 : (tt + 1) * 128, :]
                    )
                    x_tiles[tt] = x_t

                for tt in range(4):
                    load_x(tt)
                nc.sync.dma_start(
                    out=w_qkv, in_=wqkv.ap().rearrange("(cc p) n -> p cc n", p=128)
                )
                for tt in range(4, NT):
                    load_x(tt)
                nc.sync.dma_start(
                    out=w_p, in_=wp.ap().rearrange("(P p) n -> p P n", p=128)
                )
                nc.sync.dma_start(
                    out=bq_t, in_=bqkv.ap()[0:CL].rearrange("(P p) -> p P", p=128)
                )
                nc.sync.dma_start(
                    out=bk_t,
                    in_=bqkv.ap()[CL : 2 * CL].rearrange("(P p) -> p P", p=128),
                )
                if mode == "causal":
                    nc.sync.dma_start(out=maskd_t, in_=maskd[:, :])
                if vbias:
                    bv_t = pco.tile([128, CL], F32, tag="bv")
                    bv_ap = bass.AP(
                        tensor=bqkv.ap().tensor, offset=2 * CL,
                        ap=[[0, 128], [1, CL]],
                    )
                    nc.sync.dma_start(out=bv_t, in_=bv_ap)

                # ---- Phase A: LN -> bf16 xn -> DMA-xbar transpose.
                # Split into two stages emitted with a one-tile lag: the
                # DVE->Act->DVE->Act sem round-trips then overlap the
                # neighboring tile's engine work instead of blocking the
                # in-order engine queues (which would pace the pipeline at
                # the full chain latency, ~2.3us/tile).
                a_state = {}
                def phase_a_stats(tt):
                    x_t = x_tiles.pop(tt)
                    stats = lnp.tile([128, 2, 6], F32, tag="stats")
                    xg = x_t[:, :].rearrange("p (g d) -> p g d", g=2)
                    for g in range(2):
                        nc.vector.bn_stats(out=stats[:, g, :], in_=xg[:, g, :])
                    mv = lnp.tile([128, 2], F32, tag="mv")
                    nc.vector.bn_aggr(out=mv[:, :], in_=stats[:, :, :])
                    rstd = lnp.tile([128, 1], F32, tag="rstd")
                    nc.scalar.activation(
                        out=rstd[:, :], in_=mv[:, 1:2], func=Sqrt,
                        bias=eps_t[:, :], scale=1.0,
                    )
                    a_state[tt] = (x_t, mv, rstd)

                def phase_a_fin(tt):
                    if tt not in a_state:
                        return
                    x_t, mv, rstd = a_state.pop(tt)
                    nc.vector.reciprocal(out=rstd[:, :], in_=rstd[:, :])
                    nmr = lnp.tile([128, 1], F32, tag="nmr")
                    nc.gpsimd.tensor_scalar(
                        nmr[:, :], mv[:, 0:1], rstd[:, :], -1.0,
                        op0=MULT, op1=MULT,
                    )
                    # xn split Act/Pool (sbuf-only op; Pool cannot touch
                    # PSUM so it gets the sbuf work)
                    xn_t = xnp.tile([128, C], BF16, tag="xn")
                    nc.gpsimd.tensor_scalar(
                        xn_t[:, :], x_t[:, :], rstd[:, :], nmr[:, :],
                        op0=MULT, op1=ADD,
                    )
                    nc.sync.dma_start(
                        out=xnT[:, :, tt * 128 : (tt + 1) * 128],
                        in_=xn_t[:, :], transpose=True,
                    )

                # ---- Phase B: Q/K projections per 512-token group.
                # P=0 (heads 0,1) is emitted with its group so the exp stream
                # can start right after phase A; P=1 is deferred into head 1's
                # i-loop as PE filler (its drains go to Pool/DVE since Act is
                # then busy with exps).
                def emit_qk(g, which, P, on_act):
                    dest, bias_t = (qT, bq_t) if which == 0 else (kT, bk_t)
                    wcol = which * CL + P * 128
                    ps = scp.tile(
                        [128, 512], F32, tag="sc",
                        name=f"qkps_{g}_{which}_{P}",
                    )
                    for cc in range(NCC):
                        nc.tensor.matmul(
                            ps[:, :],
                            w_qkv[:, cc, wcol : wcol + 128],
                            xnT[:, cc, g * 512 : (g + 1) * 512],
                            start=(cc == 0), stop=(cc == NCC - 1),
                        )
                    if on_act:
                        nc.scalar.activation(
                            out=dest[:, P, g * 512 : (g + 1) * 512],
                            in_=ps[:, :], func=Ident,
                            bias=bias_t[:, P, :], scale=1.0,
                        )
                    else:
                        nc.vector.tensor_scalar(
                            dest[:, P, g * 512 : (g + 1) * 512],
                            ps[:, :], bias_t[:, P, :], None, op0=ADD,
                        )

                # (phase B QK emission is interleaved with phase A and early
                # head-0 score chunks below, after the helpers are defined)

                # ---- V projection (interleaved into head 0's i-loop)
                def emit_v(tt):
                    ps = vps.tile([128, 512], F32, tag="v",
                                  name=f"vps_{tt}")[:, 0:CL]
                    for cc in range(NCC):
                        nc.tensor.matmul(
                            ps[:, :],
                            xnT[:, cc, tt * 128 : (tt + 1) * 128],
                            w_qkv[:, cc, 2 * CL : 3 * CL],
                            start=(cc == 0), stop=(cc == NCC - 1),
                        )
                    psh = ps[:, :].rearrange("p (h d) -> p h d", h=HL)
                    if vbias:
                        bvh = bv_t[:, :].rearrange("p (h d) -> p h d", h=HL)
                        nc.vector.tensor_tensor(
                            out=Vp[:, tt, :, 0:D], in0=psh, in1=bvh, op=ADD
                        )
                    else:
                        nc.vector.tensor_copy(out=Vp[:, tt, :, 0:D], in_=psh)

                # ---- Phase D (emitted interleaved): output projection.
                # use_act: route drains to Act only after the exp stream ends
                npr = 0
                def emit_proj(tt, use_act):
                    nonlocal npr
                    o_t = outp.tile([128, C], BF16, tag="o", name=f"o_{tt}")
                    for n in range(2):
                        ps = scp.tile([128, 512], F32, tag="sc",
                                      name=f"prps_{tt}_{n}")
                        for P in range(2):
                            nc.tensor.matmul(
                                ps[:, :],
                                attnT[:, P, tt * 128 : (tt + 1) * 128],
                                w_p[:, P, n * 512 : (n + 1) * 512],
                                start=(P == 0), stop=(P == 1),
                            )
                        osl = o_t[:, n * 512 : (n + 1) * 512]
                        if use_act and n == 0:
                            nc.scalar.copy(out=osl, in_=ps[:, :])
                        else:
                            npr += 1
                            nc.vector.tensor_copy(out=osl, in_=ps[:, :])
                    nc.sync.dma_start(
                        out=y_out[tt * 128 : (tt + 1) * 128, :], in_=o_t[:, :]
                    )

                # ---- Phase C: attention, one flat software-pipelined slot
                # stream across all heads.  PV consumption lags scores/exp
                # production by a FULL HEAD (LAGM=16 slots) in causal mode:
                # every PV's exp dependency is then a whole head old, so PE
                # never blocks on the exp chain, and PE's surplus work (V,
                # QK-P1, proj) fills the Act-lag inside each score phase.
                LAGM = NT if causal else 2
                pv_tiles, p_tiles = {}, {}
                next_c = {}
                pending_bcast = []

                def flush_bcast():
                    """PE-side broadcast of 1/sums across partitions, emitted
                    one slot late so the DVE reciprocal has time to land."""
                    while pending_bcast:
                        h, j, rc_row = pending_bcast.pop(0)
                        P, hp = h // 2, (h % 2) * 64
                        sl = slice(512 * j, 512 * j + 512)
                        rc_ps = vps.tile([128, 512], F32, tag="v",
                                         name=f"rcps_{h}_{j}")
                        nc.tensor.matmul(
                            rc_ps[hp : hp + 64, :],
                            ones_bc[64:65, 0:64],
                            rc_row[64:65, :],
                            start=True, stop=True,
                        )
                        nc.vector.tensor_tensor(
                            out=attnT[hp : hp + 64, P, sl],
                            in0=attnT[hp : hp + 64, P, sl],
                            in1=rc_ps[hp : hp + 64, :], op=MULT,
                        )

                def emit_score_chunk(h, i):
                    """Emit ONE 512-col score chunk for (h, i).  Returns False
                    when tile i is fully emitted.  Lazily allocates the p tile
                    (exact causal width) and the head's pv psum tiles."""
                    P, hp = h // 2, (h % 2) * 64
                    base = 128 * i if causal else 0
                    width = T - base
                    c0 = next_c.get((h, i), 0)
                    if c0 >= width:
                        return False
                    if c0 == 0:
                        if causal:
                            p_t = ppool.tile([128, width], BF16, tag=f"p{i}",
                                             name=f"p_{h}_{i}")
                        else:
                            p_t = ppool.tile([128, T], BF16, tag="p",
                                             name=f"p_{h}_{i}")
                        p_tiles[(h, i)] = p_t
                    p_t = p_tiles[(h, i)]
                    w = min(512, width - c0)
                    ps = scp.tile([128, 512], F32, tag="sc",
                                  name=f"scps_{h}_{i}_{c0}")
                    nc.tensor.matmul(
                        ps[:, 0:w],
                        kT[hp : hp + D, P, i * 128 : (i + 1) * 128],
                        qT[hp : hp + D, P, base + c0 : base + c0 + w],
                        start=True, stop=True,
                    )
                    nc.scalar.activation(
                        out=p_t[:, c0 : c0 + w], in_=ps[:, 0:w],
                        func=Exp, scale=SCORE_SCALE,
                    )
                    if mode == "full":
                        m_t = mpool.tile([128, 512], BF16, tag="m")
                        nc.sync.dma_start(
                            out=m_t[:, 0:w],
                            in_=maskt[
                                i * 128 : (i + 1) * 128,
                                base + c0 : base + c0 + w,
                            ],
                        )
                        nc.vector.tensor_tensor(
                            out=p_t[:, c0 : c0 + w],
                            in0=p_t[:, c0 : c0 + w],
                            in1=m_t[:, 0:w], op=MULT,
                        )
                    if causal and c0 == 0:
                        nc.gpsimd.tensor_tensor(
                            out=p_t[:, 0:128], in0=p_t[:, 0:128],
                            in1=maskd_t[:, :], op=MULT,
                        )
                    next_c[(h, i)] = c0 + w
                    return True

                def emit_scores_all(h, i):
                    while emit_score_chunk(h, i):
                        pass

                def emit_pv(h, i):
                    P, hp = h // 2, (h % 2) * 64
                    if i == 0:
                        pv_tiles[h] = [
                            pvp.tile([65, 512], F32, tag="pv",
                                     name=f"pv_{h}_{j}")
                            for j in range(4)
                        ]
                    pv = pv_tiles[h]
                    base = 128 * i if causal else 0
                    p_t = p_tiles.pop((h, i))
                    jlo = i // 4 if causal else 0
                    for j in range(jlo, 4):
                        off = 512 * j - base   # local col in p_t
                        o0 = max(0, off)
                        skip = o0 - off        # masked lead columns
                        last = (i == 4 * j + 3) if causal else (i == NT - 1)
                        nc.tensor.matmul(
                            pv[j][:, skip : 512],
                            Vp[:, i, h, :],
                            p_t[:, o0 : off + 512],
                            start=(i == 0), stop=last,
                        )
                        if last:
                            # drain: attn^T rows to sbuf; 1/sums row via DVE
                            # (read straight from the psum sums row); the
                            # cross-partition broadcast happens on PE one
                            # slot later (flush_bcast)
                            sl = slice(512 * j, 512 * j + 512)
                            nc.vector.tensor_copy(
                                out=attnT[hp : hp + 64, P, sl],
                                in_=pv[j][0:64, :],
                            )
                            rc_row = rcp.tile([65, 512], BF16, tag="rc",
                                              name=f"rc_{h}_{j}")
                            with nc.allow_low_precision(
                                reason="f32r out is bitwise f32"
                            ):
                                nc.vector.reciprocal(
                                    out=rc_row[64:65, :], in_=pv[j][64:65, :]
                                )
                            pending_bcast.append((h, j, rc_row))

                # ---- Phases A+B (+ early head-0 score chunks in causal mode
                # so the Act exp stream starts as soon as QK-P0(g0) lands)
                for g in range(4):
                    for tt in range(4 * g, 4 * g + 4):
                        phase_a_stats(tt)
                        phase_a_fin(tt - 1)
                    phase_a_fin(4 * g + 3)
                    emit_qk(g, 0, 0, False)
                    emit_qk(g, 1, 0, False)
                    if causal:
                        cmax = 512 * (g + 1) if g < 3 else T
                        for i in range(4 * g + 4):
                            base = 128 * i
                            while True:
                                c0 = next_c.get((0, i), 0)
                                w = min(512, T - base - c0)
                                if w <= 0 or base + c0 + w > cmax:
                                    break
                                emit_score_chunk(0, i)

                # ---- main slot stream.  Causal: the whole schedule runs one
                # head-phase early (h0's scores live in phase B), so slots
                # 0-15 carry S(h1)+V, 16-31 S(h2)+PV(h0), 32-47 S(h3)+PV(h1),
                # 48-63 PV(h2), 64-79 PV(h3)+proj.
                if causal:
                    NS, SOFF, PVOFF = (HL - 1) * NT, 1, NT
                else:
                    NS, SOFF, PVOFF = HL * NT, 0, LAGM
                for g in range(HL * NT + PVOFF):
                    flush_bcast()
                    if g < NS:
                        h, i = (g // NT) + SOFF, g % NT
                        emit_scores_all(h, i)
                    if causal:
                        if g < NT:
                            emit_v(g)
                            if g in (1, 3, 5, 7):
                                emit_qk((g - 1) // 2, 0, 1, False)  # q-P1
                            elif g == 9:
                                emit_qk(0, 1, 1, False)             # k-P1 g0
                        elif g in (17, 21, 25):
                            emit_qk((g - 13) // 4, 1, 1, False)     # k-P1 g1-3
                    elif g < NS:
                        if h == 0:
                            emit_v(i)
                        elif h == 1 and i in (1, 3, 5, 7):
                            emit_qk((i - 1) // 2, 0, 1, False)
                        elif h == 1 and i == 9:
                            emit_qk(0, 1, 1, False)
                        elif h == 2 and i in (1, 5, 9):
                            emit_qk(i // 4 + 1, 1, 1, False)
                    if g >= PVOFF:
                        h2, i2 = divmod(g - PVOFF, NT)
                        emit_pv(h2, i2)
                    # proj groups interleave into the PV tail once their
                    # attn^T j-chunks are fully scaled
                    if causal and g in (70, 74, 78):
                        j = (g - 70) // 4
                        for tt in range(4 * j, 4 * j + 4):
                            emit_proj(tt, True)
                flush_bcast()

                for tt in range(12 if causal else 0, NT):
                    emit_proj(tt, True)

    _split_waits(nc, limit=1)
    return nc


def kernel(x, mask, ln_scale, ln_bias, qkv_w, qkv_b, proj_w, proj_b):
    import ml_dtypes

    bf = ml_dtypes.bfloat16
    x = np.ascontiguousarray(np.asarray(x), dtype=np.float32)
    mask2 = np.asarray(mask).reshape(T, T)
    ln_scale = np.asarray(ln_scale, dtype=np.float32)
    ln_bias = np.asarray(ln_bias, dtype=np.float32)
    qkv_w = np.asarray(qkv_w, dtype=np.float32)
    qkv_b = np.asarray(qkv_b, dtype=np.float32)
    proj_w = np.asarray(proj_w, dtype=np.float32)
    proj_b = np.asarray(proj_b, dtype=np.float32)

    # fold LayerNorm affine into the qkv projection (exact host-side algebra)
    w_eff = (ln_scale[:, None] * qkv_w).astype(np.float32)
    b_eff = (ln_bias @ qkv_w + qkv_b).astype(np.float32)

    if mask2.all():
        mode = "none"
    elif np.array_equal(mask2, np.tril(np.ones((T, T), dtype=mask2.dtype))):
        mode = "causal"
    else:
        mode = "full"

    in_maps = []
    core_ids = list(range(8))
    vbias = bool(np.any(b_eff[2 * C : 3 * C] != 0.0))
    maskt_f = None
    maskd_m = None
    if mode == "causal":
        # diag strip mask: maskT[k, q] for the 128x128 diagonal block
        maskd_m = np.ascontiguousarray(mask2[0:128, 0:128].T.astype(bf))
    elif mode == "full":
        maskt_f = np.ascontiguousarray(mask2.T.astype(bf))

    for core in core_ids:
        b = core // 4
        hs = 4 * (core % 4)
        cols_q = slice(hs * D, hs * D + CL)
        cols_k = slice(C + hs * D, C + hs * D + CL)
        cols_v = slice(2 * C + hs * D, 2 * C + hs * D + CL)
        wl = np.concatenate(
            [w_eff[:, cols_q], w_eff[:, cols_k], w_eff[:, cols_v]], axis=1
        )
        bl = np.concatenate([b_eff[cols_q], b_eff[cols_k], b_eff[cols_v]])
        im = {
            "x_b": np.ascontiguousarray(x[b].astype(bf)),
            "wqkv": np.ascontiguousarray(wl.astype(bf)),
            "bqkv": np.ascontiguousarray(bl),
            "wp": np.ascontiguousarray(
                proj_w[hs * D : hs * D + CL, :].astype(bf)
            ),
        }
        if mode == "causal":
            im["maskd"] = maskd_m
        elif mode == "full":
            im["maskt"] = maskt_f
        in_maps.append(im)

    nc = _build(mode, vbias)
    trace = bool(int(os.environ.get("KERNEL_TRACE", "0")))
    res = run_bass_kernel_spmd(nc, in_maps, core_ids=core_ids, trace=trace)
    global LAST_RESULT
    LAST_RESULT = res

    out = np.zeros((B, T, C), dtype=np.float32)
    for core in core_ids:
        out[core // 4] += res.results[core]["y_part"].astype(np.float32)
    out += proj_b[None, None, :]
    return out


if __name__ == "__main__":
    rng = np.random.default_rng(0)
    x = rng.standard_normal((B, T, C), dtype=np.float32)
    mask = np.tril(np.ones((T, T), dtype=bool))[None, None]
    ln_scale = np.ones(C, np.float32)
    ln_bias = np.zeros(C, np.float32)
    lim = float(np.sqrt(6.0 / (C + 3 * C)))
    qkv_w = rng.uniform(-lim, lim, (C, 3 * C)).astype(np.float32)
    qkv_b = np.zeros(3 * C, np.float32)
    limp = float(np.sqrt(6.0 / (C + C)))
    proj_w = rng.uniform(-limp, limp, (C, C)).astype(np.float32)
    proj_b = np.zeros(C, np.float32)
    out = kernel(x, mask, ln_scale, ln_bias, qkv_w, qkv_b, proj_w, proj_b)
    print("out", out.shape, out.dtype, np.abs(out).max())


# revision 82
# speedup vs baseline: 1.6669x; 1.0172x over previous
"""Trainium2 Bass kernel for nn_Attention (pre-LN causal attention block).

Reference computation (B=2, T=2048, C=1024, H=16, D=64, fp32):
    xn = LayerNorm(x)                       (eps=1e-6)
    qkv = xn @ qkv_w + qkv_b;  q,k,v = split(qkv)
    scores = (q @ k^T) / sqrt(D), causal-masked, softmax
    out = (softmax @ v) reshaped @ proj_w + proj_b

Sharding (8 cores): data-parallel over B (cores 0-3 <- batch 0, 4-7 <- batch 1)
x tensor-parallel over heads (4 heads/core: qkv_w column-sharded, proj_w
row-sharded).  Each core emits a partial projection output; the host sums the
4 partials per batch and adds proj_b (the "all-reduce after proj" done
host-side).

Device kernel design notes (measured 168us/core cost-model vs 275us baseline):
  - All matmuls in bf16 (f32 psum accumulation); bf16 runs at full PE rate at
    ANY free size (fp32r needs >=256), enabling exact causal granularity:
    scores^T tiles for k-tile i cover q in [128i, T) -> 17408 rows/head.
  - Scores are computed TRANSPOSED (s^T[k,q] = K tile @ Q^T) so the PV matmul
    consumes softmax tiles directly: pv[j] += V'[i]^T @ p_i with V' augmented
    by a ones column (row 64 of the psum accumulates sum(p) for free).
    PV output [65, q] rows 0..63 are attn^T -- exactly the proj lhsT layout.
  - Matmul instruction count is kept low (~660): each InstMatmult costs
    ~125ns of PE sequencer decode (Ldweights+Matmult), which rate-limits
    designs with many small matmuls regardless of engine time.
  - Softmax denominators: DVE reciprocal reads the psum sums row in place;
    a 1-partition-contraction PE matmul (ones[1,64] x recip_row[1,512], bf16)
    broadcasts it across partitions into psum; DVE scales attn^T in place.
    No DRAM bounce, no cross-engine round trips on the in-order queues.
  - LayerNorm: bn_stats/bn_aggr (DVE) + Sqrt (Act) split into two stages
    emitted one tile apart so cross-engine sem round-trips overlap the
    neighboring tile; apply as per-partition scale/bias on Act/Pool -> bf16
    xn; one 2-byte DMA-xbar transpose per tile gives xn^T.  x is staged bf16
    (host cast, halves x DMA); y partial is bf16 (host f32 sum).
  - Global software pipeline: ALL x loads are issued up front (a DMA dispatch
    that waits holds its queue, so transposes must never precede loads);
    head 0's scores run inside the LN/QKV phase; PV lags scores by a full
    head-phase (exp deps are ~16 slots old when PE consumes them); V, QK-P1
    and the output projection fill PE slack inside later score phases.
  - Engine split obeys "GPSIMD cannot access PSUM": Pool gets sbuf-only work
    (LN apply, diag mask), Act owns the exp stream, DVE takes psum drains.
  - Validated numerics (numpy bf16 simulation of this cast structure):
    rel err ~5e-3 vs tolerance 2e-2; measured on HW: 4.9e-3.
"""

import os

import numpy as np

import concourse.bass as bass
import concourse.tile as tile
from concourse import mybir
from concourse.bass_utils import run_bass_kernel_spmd

LAST_RESULT = None
F32 = mybir.dt.float32
F32R = mybir.dt.float32r
BF16 = mybir.dt.bfloat16

B, T, C = 2, 2048, 1024
H, D = 16, 64
HL = 4            # heads per core
CL = HL * D       # local c-dim (256)
NT = T // 128     # 16 token tiles
NCC = C // 128    # 8 contraction chunks
LN_EPS = 1e-6
SCORE_SCALE = 0.125  # 1/sqrt(D)/TEMP


def _split_waits(nc, limit=1):
    """This container's walrus rejects instructions with >1 sem wait ("Too many
    sync wait commands").  Move excess waits onto same-engine NOPs inserted
    just before the instruction (equivalent under per-engine program order)."""
    n = 0
    for f in nc.m.functions:
        for b in f.blocks:
            insts = b.instructions
            if not any(
                i.sync_info is not None and len(i.sync_info.on_wait) > limit
                for i in insts
            ):
                continue
            new = []
            for inst in insts:
                si = inst.sync_info
                if si is not None and len(si.on_wait) > limit:
                    waits = list(si.on_wait)
                    excess, keep = waits[:-limit], waits[-limit:]
                    for j in range(0, len(excess), limit):
                        n += 1
                        nop = mybir.InstNoOp(name=f"I-wsplit-{n}", ins=[], outs=[])
                        nop.engine = inst.engine
                        nop.sync_info = mybir.SyncInfo(
                            on_wait=excess[j : j + limit], on_update=[]
                        )
                        new.append(nop)
                    inst.sync_info = mybir.SyncInfo(
                        on_wait=keep, on_update=list(si.on_update)
                    )
                new.append(inst)
            b.instructions = new
    return n


def _build(mode, vbias):
    """mode: 'causal' (tile-skip + diag mask), 'none' (no mask), 'full'
    (arbitrary mask, maskT input).  vbias: apply per-column v bias."""
    causal = mode == "causal"
    nc = bass.Bass(name="attnblk")
    x_in = nc.declare_dram_parameter("x_b", [T, C], BF16, isOutput=False)
    wqkv = nc.declare_dram_parameter("wqkv", [C, 3 * CL], BF16, isOutput=False)
    bqkv = nc.declare_dram_parameter("bqkv", [3 * CL], F32, isOutput=False)
    wp = nc.declare_dram_parameter("wp", [CL, C], BF16, isOutput=False)
    if mode == "causal":
        maskd = nc.declare_dram_parameter("maskd", [128, 128], BF16, isOutput=False)
    elif mode == "full":
        maskt = nc.declare_dram_parameter("maskt", [T, T], BF16, isOutput=False)
    y_out = nc.declare_dram_parameter("y_part", [T, C], BF16, isOutput=True)

    Exp = mybir.ActivationFunctionType.Exp
    Sqrt = mybir.ActivationFunctionType.Sqrt
    Ident = mybir.ActivationFunctionType.Identity
    MULT = mybir.AluOpType.mult
    ADD = mybir.AluOpType.add
    SUBTRACT = mybir.AluOpType.subtract

    with tile.TileContext(nc) as tc:
        with (
            tc.tile_pool(name="persist", bufs=1) as pp,
            tc.tile_pool(name="small", bufs=1) as pco,
        ):
            # ---- persistent sbuf tensors
            xnT = pp.tile([128, NCC, T], BF16, tag="xnT")
            qT = pp.tile([128, 2, T], BF16, tag="qT")      # pair-stacked Q^T
            kT = pp.tile([128, 2, T], BF16, tag="kT")
            Vp = pp.tile([128, NT, HL, D + 1], BF16, tag="Vp")  # V' + ones col
            w_qkv = pp.tile([128, NCC, 3 * CL], BF16, tag="wqkv")
            w_p = pp.tile([128, 2, C], BF16, tag="w_p")
            attnT = pp.tile([128, 2, T], BF16, tag="attnT")
            eps_t = pco.tile([128, 1], F32, tag="eps")
            bq_t = pco.tile([128, 2, 1], F32, tag="bq")
            bk_t = pco.tile([128, 2, 1], F32, tag="bk")
            ones_bc = pco.tile([65, D], BF16, tag="ones_bc")
            if mode == "causal":
                maskd_t = pco.tile([128, 128], BF16, tag="maskd")

            nc.vector.memset(eps_t, LN_EPS)
            nc.vector.memset(ones_bc[:, :], 1.0)
            # ones columns for the sums row; V drains overwrite cols 0..D-1
            nc.vector.memset(Vp[:, :, :, :], 1.0)

            with (
                tc.tile_pool(name="sc_ps", bufs=3, space="PSUM") as scp,
                tc.tile_pool(name="v_ps", bufs=1, space="PSUM") as vps,
                tc.tile_pool(name="pv_ps", bufs=4, space="PSUM") as pvp,
                tc.tile_pool(name="x_pool", bufs=16) as xp,
                tc.tile_pool(name="ln_pool", bufs=4) as lnp,
                tc.tile_pool(name="xn_pool", bufs=4) as xnp,
                tc.tile_pool(name="p_pool", bufs=2 if causal else 4) as ppool,
                tc.tile_pool(name="rc_pool", bufs=4) as rcp,
                tc.tile_pool(name="m_pool", bufs=2) as mpool,
                tc.tile_pool(name="out_pool", bufs=4) as outp,
            ):
                x_tiles = {}
                def load_x(tt):
                    x_t = xp.tile([128, C], BF16, tag="x", name=f"x_{tt}")
                    nc.sync.dma_start(
                        out=x_t, in_=x_in[tt * 128 : (tt + 1) * 128, :]
                    )
                    x_tiles[tt] = x_t

                for tt in range(4):
                    load_x(tt)
                nc.sync.dma_start(
                    out=w_qkv, in_=wqkv.ap().rearrange("(cc p) n -> p cc n", p=128)
                )
                for tt in range(4, NT):
                    load_x(tt)
                nc.sync.dma_start(
                    out=w_p, in_=wp.ap().rearrange("(P p) n -> p P n", p=128)
                )
                nc.sync.dma_start(
                    out=bq_t, in_=bqkv.ap()[0:CL].rearrange("(P p) -> p P", p=128)
                )
                nc.sync.dma_start(
                    out=bk_t,
                    in_=bqkv.ap()[CL : 2 * CL].rearrange("(P p) -> p P", p=128),
                )
                if mode == "causal":
                    nc.sync.dma_start(out=maskd_t, in_=maskd[:, :])
                if vbias:
                    bv_t = pco.tile([128, CL], F32, tag="bv")
                    bv_ap = bass.AP(
                        tensor=bqkv.ap().tensor, offset=2 * CL,
                        ap=[[0, 128], [1, CL]],
                    )
                    nc.sync.dma_start(out=bv_t, in_=bv_ap)

                # ---- Phase A: LN -> bf16 xn -> DMA-xbar transpose.
                # Split into two stages emitted with a one-tile lag: the
                # DVE->Act->DVE->Act sem round-trips then overlap the
                # neighboring tile's engine work instead of blocking the
                # in-order engine queues (which would pace the pipeline at
                # the full chain latency, ~2.3us/tile).
                a_state = {}
                def phase_a_stats(tt):
                    x_t = x_tiles.pop(tt)
                    stats = lnp.tile([128, 2, 6], F32, tag="stats")
                    xg = x_t[:, :].rearrange("p (g d) -> p g d", g=2)
                    for g in range(2):
                        nc.vector.bn_stats(out=stats[:, g, :], in_=xg[:, g, :])
                    mv = lnp.tile([128, 2], F32, tag="mv")
                    nc.vector.bn_aggr(out=mv[:, :], in_=stats[:, :, :])
                    rstd = lnp.tile([128, 1], F32, tag="rstd")
                    nc.scalar.activation(
                        out=rstd[:, :], in_=mv[:, 1:2], func=Sqrt,
                        bias=eps_t[:, :], scale=1.0,
                    )
                    a_state[tt] = (x_t, mv, rstd)

                def phase_a_fin(tt):
                    if tt not in a_state:
                        return
                    x_t, mv, rstd = a_state.pop(tt)
                    nc.vector.reciprocal(out=rstd[:, :], in_=rstd[:, :])
                    nmr = lnp.tile([128, 1], F32, tag="nmr")
                    nc.gpsimd.tensor_scalar(
                        nmr[:, :], mv[:, 0:1], rstd[:, :], -1.0,
                        op0=MULT, op1=MULT,
                    )
                    # xn split Act/Pool (sbuf-only op; Pool cannot touch
                    # PSUM so it gets the sbuf work)
                    xn_t = xnp.tile([128, C], BF16, tag="xn")
                    nc.gpsimd.tensor_scalar(
                        xn_t[:, :], x_t[:, :], rstd[:, :], nmr[:, :],
                        op0=MULT, op1=ADD,
                    )
                    nc.sync.dma_start(
                        out=xnT[:, :, tt * 128 : (tt + 1) * 128],
                        in_=xn_t[:, :], transpose=True,
                    )

                # ---- Phase B: Q/K projections per 512-token group.
                # P=0 (heads 0,1) is emitted with its group so the exp stream
                # can start right after phase A; P=1 is deferred into head 1's
                # i-loop as PE filler (its drains go to Pool/DVE since Act is
                # then busy with exps).
                def emit_qk(g, which, P, on_act):
                    dest, bias_t = (qT, bq_t) if which == 0 else (kT, bk_t)
                    wcol = which * CL + P * 128
                    ps = scp.tile(
                        [128, 512], F32, tag="sc",
                        name=f"qkps_{g}_{which}_{P}",
                    )
                    for cc in range(NCC):
                        nc.tensor.matmul(
                            ps[:, :],
                            w_qkv[:, cc, wcol : wcol + 128],
                            xnT[:, cc, g * 512 : (g + 1) * 512],
                            start=(cc == 0), stop=(cc == NCC - 1),
                        )
                    if on_act:
                        nc.scalar.activation(
                            out=dest[:, P, g * 512 : (g + 1) * 512],
                            in_=ps[:, :], func=Ident,
                            bias=bias_t[:, P, :], scale=1.0,
                        )
                    else:
                        nc.vector.tensor_scalar(
                            dest[:, P, g * 512 : (g + 1) * 512],
                            ps[:, :], bias_t[:, P, :], None, op0=ADD,
                        )

                # (phase B QK emission is interleaved with phase A and early
                # head-0 score chunks below, after the helpers are defined)

                # ---- V projection (interleaved into head 0's i-loop)
                def emit_v(tt):
                    ps = vps.tile([128, 512], F32, tag="v",
                                  name=f"vps_{tt}")[:, 0:CL]
                    for cc in range(NCC):
                        nc.tensor.matmul(
                            ps[:, :],
                            xnT[:, cc, tt * 128 : (tt + 1) * 128],
                            w_qkv[:, cc, 2 * CL : 3 * CL],
                            start=(cc == 0), stop=(cc == NCC - 1),
                        )
                    psh = ps[:, :].rearrange("p (h d) -> p h d", h=HL)
                    if vbias:
                        bvh = bv_t[:, :].rearrange("p (h d) -> p h d", h=HL)
                        nc.vector.tensor_tensor(
                            out=Vp[:, tt, :, 0:D], in0=psh, in1=bvh, op=ADD
                        )
                    else:
                        nc.vector.tensor_copy(out=Vp[:, tt, :, 0:D], in_=psh)

                # ---- Phase D (emitted interleaved): output projection.
                # use_act: route drains to Act only after the exp stream ends
                npr = 0
                def emit_proj(tt, use_act):
                    nonlocal npr
                    o_t = outp.tile([128, C], BF16, tag="o", name=f"o_{tt}")
                    for n in range(2):
                        ps = scp.tile([128, 512], F32, tag="sc",
                                      name=f"prps_{tt}_{n}")
                        for P in range(2):
                            nc.tensor.matmul(
                                ps[:, :],
                                attnT[:, P, tt * 128 : (tt + 1) * 128],
                                w_p[:, P, n * 512 : (n + 1) * 512],
                                start=(P == 0), stop=(P == 1),
                            )
                        osl = o_t[:, n * 512 : (n + 1) * 512]
                        if use_act and n == 0:
                            nc.scalar.copy(out=osl, in_=ps[:, :])
                        else:
                            npr += 1
                            nc.vector.tensor_copy(out=osl, in_=ps[:, :])
                    nc.sync.dma_start(
                        out=y_out[tt * 128 : (tt + 1) * 128, :], in_=o_t[:, :]
                    )

                # ---- Phase C: attention, one flat software-pipelined slot
                # stream across all heads.  PV consumption lags scores/exp
                # production by a FULL HEAD (LAGM=16 slots) in causal mode:
                # every PV's exp dependency is then a whole head old, so PE
                # never blocks on the exp chain, and PE's surplus work (V,
                # QK-P1, proj) fills the Act-lag inside each score phase.
                LAGM = NT if causal else 2
                pv_tiles, p_tiles = {}, {}
                next_c = {}
                pending_bcast = []

                def flush_bcast():
                    """PE-side broadcast of 1/sums across partitions, emitted
                    one slot late so the DVE reciprocal has time to land."""
                    while pending_bcast:
                        h, j, rc_row = pending_bcast.pop(0)
                        P, hp = h // 2, (h % 2) * 64
                        sl = slice(512 * j, 512 * j + 512)
                        rc_ps = vps.tile([128, 512], F32, tag="v",
                                         name=f"rcps_{h}_{j}")
                        nc.tensor.matmul(
                            rc_ps[hp : hp + 64, :],
                            ones_bc[64:65, 0:64],
                            rc_row[64:65, :],
                            start=True, stop=True,
                        )
                        nc.vector.tensor_tensor(
                            out=attnT[hp : hp + 64, P, sl],
                            in0=attnT[hp : hp + 64, P, sl],
                            in1=rc_ps[hp : hp + 64, :], op=MULT,
                        )

                def emit_score_chunk(h, i):
                    """Emit ONE 512-col score chunk for (h, i).  Returns False
                    when tile i is fully emitted.  Lazily allocates the p tile
                    (exact causal width) and the head's pv psum tiles."""
                    P, hp = h // 2, (h % 2) * 64
                    base = 128 * i if causal else 0
                    width = T - base
                    c0 = next_c.get((h, i), 0)
                    if c0 >= width:
                        return False
                    if c0 == 0:
                        if causal:
                            p_t = ppool.tile([128, width], BF16, tag=f"p{i}",
                                             name=f"p_{h}_{i}")
                        else:
                            p_t = ppool.tile([128, T], BF16, tag="p",
                                             name=f"p_{h}_{i}")
                        p_tiles[(h, i)] = p_t
                    p_t = p_tiles[(h, i)]
                    w = min(512, width - c0)
                    ps = scp.tile([128, 512], F32, tag="sc",
                                  name=f"scps_{h}_{i}_{c0}")
                    nc.tensor.matmul(
                        ps[:, 0:w],
                        kT[hp : hp + D, P, i * 128 : (i + 1) * 128],
                        qT[hp : hp + D, P, base + c0 : base + c0 + w],
                        start=True, stop=True,
                    )
                    nc.scalar.activation(
                        out=p_t[:, c0 : c0 + w], in_=ps[:, 0:w],
                        func=Exp, scale=SCORE_SCALE,
                    )
                    if mode == "full":
                        m_t = mpool.tile([128, 512], BF16, tag="m")
                        nc.sync.dma_start(
                            out=m_t[:, 0:w],
                            in_=maskt[
                                i * 128 : (i + 1) * 128,
                                base + c0 : base + c0 + w,
                            ],
                        )
                        nc.vector.tensor_tensor(
                            out=p_t[:, c0 : c0 + w],
                            in0=p_t[:, c0 : c0 + w],
                            in1=m_t[:, 0:w], op=MULT,
                        )
                    if causal and c0 == 0:
                        nc.gpsimd.tensor_tensor(
                            out=p_t[:, 0:128], in0=p_t[:, 0:128],
                            in1=maskd_t[:, :], op=MULT,
                        )
                    next_c[(h, i)] = c0 + w
                    return True

                def emit_scores_all(h, i):
                    while emit_score_chunk(h, i):
                        pass

                def emit_pv(h, i):
                    P, hp = h // 2, (h % 2) * 64
                    if i == 0:
                        pv_tiles[h] = [
                            pvp.tile([65, 512], F32, tag="pv",
                                     name=f"pv_{h}_{j}")
                            for j in range(4)
                        ]
                    pv = pv_tiles[h]
                    base = 128 * i if causal else 0
                    p_t = p_tiles.pop((h, i))
                    jlo = i // 4 if causal else 0
                    for j in range(jlo, 4):
                        off = 512 * j - base   # local col in p_t
                        o0 = max(0, off)
                        skip = o0 - off        # masked lead columns
                        last = (i == 4 * j + 3) if causal else (i == NT - 1)
                        nc.tensor.matmul(
                            pv[j][:, skip : 512],
                            Vp[:, i, h, :],
                            p_t[:, o0 : off + 512],
                            start=(i == 0), stop=last,
                        )
                        if last:
                            # drain: attn^T rows to sbuf; 1/sums row via DVE
                            # (read straight from the psum sums row); the
                            # cross-partition broadcast happens on PE one
                            # slot later (flush_bcast)
                            sl = slice(512 * j, 512 * j + 512)
                            nc.vector.tensor_copy(
                                out=attnT[hp : hp + 64, P, sl],
                                in_=pv[j][0:64, :],
                            )
                            rc_row = rcp.tile([65, 512], BF16, tag="rc",
                                              name=f"rc_{h}_{j}")
                            with nc.allow_low_precision(
                                reason="f32r out is bitwise f32"
                            ):
                                nc.vector.reciprocal(
                                    out=rc_row[64:65, :], in_=pv[j][64:65, :]
                                )
                            pending_bcast.append((h, j, rc_row))

                # ---- Phases A+B (+ early head-0 score chunks in causal mode
                # so the Act exp stream starts as soon as QK-P0(g0) lands)
                for g in range(4):
                    for tt in range(4 * g, 4 * g + 4):
                        phase_a_stats(tt)
                        phase_a_fin(tt - 1)
                    phase_a_fin(4 * g + 3)
                    emit_qk(g, 0, 0, False)
                    emit_qk(g, 1, 0, False)
                    if causal:
                        cmax = 512 * (g + 1) if g < 3 else T
                        for i in range(4 * g + 4):
                            base = 128 * i
                            while True:
                                c0 = next_c.get((0, i), 0)
                                w = min(512, T - base - c0)
                                if w <= 0 or base + c0 + w > cmax:
                                    break
                                emit_score_chunk(0, i)

                # ---- main slot stream.  Causal: the whole schedule runs one
                # head-phase early (h0's scores live in phase B), so slots
                # 0-15 carry S(h1)+V, 16-31 S(h2)+PV(h0), 32-47 S(h3)+PV(h1),
                # 48-63 PV(h2), 64-79 PV(h3)+proj.
                if causal:
                    NS, SOFF, PVOFF = (HL - 1) * NT, 1, NT
                else:
                    NS, SOFF, PVOFF = HL * NT, 0, LAGM
                for g in range(HL * NT + PVOFF):
                    flush_bcast()
                    if g < NS:
                        h, i = (g // NT) + SOFF, g % NT
                        emit_scores_all(h, i)
                    if causal:
                        if g < NT:
                            emit_v(g)
                            if g in (1, 3, 5, 7):
                                emit_qk((g - 1) // 2, 0, 1, False)  # q-P1
                            elif g == 9:
                                emit_qk(0, 1, 1, False)             # k-P1 g0
                        elif g in (17, 21, 25):
                            emit_qk((g - 13) // 4, 1, 1, False)     # k-P1 g1-3
                    elif g < NS:
                        if h == 0:
                            emit_v(i)
                        elif h == 1 and i in (1, 3, 5, 7):
                            emit_qk((i - 1) // 2, 0, 1, False)
                        elif h == 1 and i == 9:
                            emit_qk(0, 1, 1, False)
                        elif h == 2 and i in (1, 5, 9):
                            emit_qk(i // 4 + 1, 1, 1, False)
                    if g >= PVOFF:
                        h2, i2 = divmod(g - PVOFF, NT)
                        emit_pv(h2, i2)
                    # proj groups interleave into the PV tail once their
                    # attn^T j-chunks are fully scaled
                    if causal and g in (70, 74, 78):
                        j = (g - 70) // 4
                        for tt in range(4 * j, 4 * j + 4):
                            emit_proj(tt, True)
                flush_bcast()

                for tt in range(12 if causal else 0, NT):
                    emit_proj(tt, True)

    _split_waits(nc, limit=1)
    return nc


def kernel(x, mask, ln_scale, ln_bias, qkv_w, qkv_b, proj_w, proj_b):
    import ml_dtypes

    bf = ml_dtypes.bfloat16
    x = np.ascontiguousarray(np.asarray(x), dtype=np.float32)
    mask2 = np.asarray(mask).reshape(T, T)
    ln_scale = np.asarray(ln_scale, dtype=np.float32)
    ln_bias = np.asarray(ln_bias, dtype=np.float32)
    qkv_w = np.asarray(qkv_w, dtype=np.float32)
    qkv_b = np.asarray(qkv_b, dtype=np.float32)
    proj_w = np.asarray(proj_w, dtype=np.float32)
    proj_b = np.asarray(proj_b, dtype=np.float32)

    # fold LayerNorm affine into the qkv projection (exact host-side algebra)
    w_eff = (ln_scale[:, None] * qkv_w).astype(np.float32)
    b_eff = (ln_bias @ qkv_w + qkv_b).astype(np.float32)

    if mask2.all():
        mode = "none"
    elif np.array_equal(mask2, np.tril(np.ones((T, T), dtype=mask2.dtype))):
        mode = "causal"
    else:
        mode = "full"

    in_maps = []
    core_ids = list(range(8))
    vbias = bool(np.any(b_eff[2 * C : 3 * C] != 0.0))
    maskt_f = None
    maskd_m = None
    if mode == "causal":
        # diag strip mask: maskT[k, q] for the 128x128 diagonal block
        maskd_m = np.ascontiguousarray(mask2[0:128, 0:128].T.astype(bf))
    elif mode == "full":
        maskt_f = np.ascontiguousarray(mask2.T.astype(bf))

    for core in core_ids:
        b = core // 4
        hs = 4 * (core % 4)
        cols_q = slice(hs * D, hs * D + CL)
        cols_k = slice(C + hs * D, C + hs * D + CL)
        cols_v = slice(2 * C + hs * D, 2 * C + hs * D + CL)
        wl = np.concatenate(
            [w_eff[:, cols_q], w_eff[:, cols_k], w_eff[:, cols_v]], axis=1
        )
        bl = np.concatenate([b_eff[cols_q], b_eff[cols_k], b_eff[cols_v]])
        im = {
            "x_b": np.ascontiguousarray(x[b].astype(bf)),
            "wqkv": np.ascontiguousarray(wl.astype(bf)),
            "bqkv": np.ascontiguousarray(bl),
            "wp": np.ascontiguousarray(
                proj_w[hs * D : hs * D + CL, :].astype(bf)
            ),
        }
        if mode == "causal":
            im["maskd"] = maskd_m
        elif mode == "full":
            im["maskt"] = maskt_f
        in_maps.append(im)

    nc = _build(mode, vbias)
    trace = bool(int(os.environ.get("KERNEL_TRACE", "0")))
    res = run_bass_kernel_spmd(nc, in_maps, core_ids=core_ids, trace=trace)
    global LAST_RESULT
    LAST_RESULT = res

    out = np.zeros((B, T, C), dtype=np.float32)
    for core in core_ids:
        out[core // 4] += res.results[core]["y_part"].astype(np.float32)
    out += proj_b[None, None, :]
    return out


if __name__ == "__main__":
    rng = np.random.default_rng(0)
    x = rng.standard_normal((B, T, C), dtype=np.float32)
    mask = np.tril(np.ones((T, T), dtype=bool))[None, None]
    ln_scale = np.ones(C, np.float32)
    ln_bias = np.zeros(C, np.float32)
    lim = float(np.sqrt(6.0 / (C + 3 * C)))
    qkv_w = rng.uniform(-lim, lim, (C, 3 * C)).astype(np.float32)
    qkv_b = np.zeros(3 * C, np.float32)
    limp = float(np.sqrt(6.0 / (C + C)))
    proj_w = rng.uniform(-limp, limp, (C, C)).astype(np.float32)
    proj_b = np.zeros(C, np.float32)
    out = kernel(x, mask, ln_scale, ln_bias, qkv_w, qkv_b, proj_w, proj_b)
    print("out", out.shape, out.dtype, np.abs(out).max())


# revision 83
# speedup vs baseline: 1.6753x; 1.0050x over previous
"""Trainium2 Bass kernel for nn_Attention (pre-LN causal attention block).

Reference computation (B=2, T=2048, C=1024, H=16, D=64, fp32):
    xn = LayerNorm(x)                       (eps=1e-6)
    qkv = xn @ qkv_w + qkv_b;  q,k,v = split(qkv)
    scores = (q @ k^T) / sqrt(D), causal-masked, softmax
    out = (softmax @ v) reshaped @ proj_w + proj_b

Sharding (8 cores): data-parallel over B (cores 0-3 <- batch 0, 4-7 <- batch 1)
x tensor-parallel over heads (4 heads/core: qkv_w column-sharded, proj_w
row-sharded).  Each core emits a partial projection output; the host sums the
4 partials per batch and adds proj_b (the "all-reduce after proj" done
host-side).

Device kernel design notes (measured 168us/core cost-model vs 275us baseline):
  - All matmuls in bf16 (f32 psum accumulation); bf16 runs at full PE rate at
    ANY free size (fp32r needs >=256), enabling exact causal granularity:
    scores^T tiles for k-tile i cover q in [128i, T) -> 17408 rows/head.
  - Scores are computed TRANSPOSED (s^T[k,q] = K tile @ Q^T) so the PV matmul
    consumes softmax tiles directly: pv[j] += V'[i]^T @ p_i with V' augmented
    by a ones column (row 64 of the psum accumulates sum(p) for free).
    PV output [65, q] rows 0..63 are attn^T -- exactly the proj lhsT layout.
  - Matmul instruction count is kept low (~660): each InstMatmult costs
    ~125ns of PE sequencer decode (Ldweights+Matmult), which rate-limits
    designs with many small matmuls regardless of engine time.
  - Softmax denominators: DVE reciprocal reads the psum sums row in place;
    a 1-partition-contraction PE matmul (ones[1,64] x recip_row[1,512], bf16)
    broadcasts it across partitions into psum; DVE scales attn^T in place.
    No DRAM bounce, no cross-engine round trips on the in-order queues.
  - LayerNorm: bn_stats/bn_aggr (DVE) + Sqrt (Act) split into two stages
    emitted one tile apart so cross-engine sem round-trips overlap the
    neighboring tile; apply as per-partition scale/bias on Act/Pool -> bf16
    xn; one 2-byte DMA-xbar transpose per tile gives xn^T.  x is staged bf16
    (host cast, halves x DMA); y partial is bf16 (host f32 sum).
  - Global software pipeline: ALL x loads are issued up front (a DMA dispatch
    that waits holds its queue, so transposes must never precede loads);
    head 0's scores run inside the LN/QKV phase; PV lags scores by a full
    head-phase (exp deps are ~16 slots old when PE consumes them); V, QK-P1
    and the output projection fill PE slack inside later score phases.
  - Engine split obeys "GPSIMD cannot access PSUM": Pool gets sbuf-only work
    (LN apply, diag mask), Act owns the exp stream, DVE takes psum drains.
  - Validated numerics (numpy bf16 simulation of this cast structure):
    rel err ~5e-3 vs tolerance 2e-2; measured on HW: 4.9e-3.
"""

import os

import numpy as np

import concourse.bass as bass
import concourse.tile as tile
from concourse import mybir
from concourse.bass_utils import run_bass_kernel_spmd

LAST_RESULT = None
F32 = mybir.dt.float32
F32R = mybir.dt.float32r
BF16 = mybir.dt.bfloat16

B, T, C = 2, 2048, 1024
H, D = 16, 64
HL = 4            # heads per core
CL = HL * D       # local c-dim (256)
NT = T // 128     # 16 token tiles
NCC = C // 128    # 8 contraction chunks
LN_EPS = 1e-6
SCORE_SCALE = 0.125  # 1/sqrt(D)/TEMP


def _split_waits(nc, limit=1):
    """This container's walrus rejects instructions with >1 sem wait ("Too many
    sync wait commands").  Move excess waits onto same-engine NOPs inserted
    just before the instruction (equivalent under per-engine program order)."""
    n = 0
    for f in nc.m.functions:
        for b in f.blocks:
            insts = b.instructions
            if not any(
                i.sync_info is not None and len(i.sync_info.on_wait) > limit
                for i in insts
            ):
                continue
            new = []
            for inst in insts:
                si = inst.sync_info
                if si is not None and len(si.on_wait) > limit:
                    waits = list(si.on_wait)
                    excess, keep = waits[:-limit], waits[-limit:]
                    for j in range(0, len(excess), limit):
                        n += 1
                        nop = mybir.InstNoOp(name=f"I-wsplit-{n}", ins=[], outs=[])
                        nop.engine = inst.engine
                        nop.sync_info = mybir.SyncInfo(
                            on_wait=excess[j : j + limit], on_update=[]
                        )
                        new.append(nop)
                    inst.sync_info = mybir.SyncInfo(
                        on_wait=keep, on_update=list(si.on_update)
                    )
                new.append(inst)
            b.instructions = new
    return n


def _build(mode, vbias):
    """mode: 'causal' (tile-skip + diag mask), 'none' (no mask), 'full'
    (arbitrary mask, maskT input).  vbias: apply per-column v bias."""
    causal = mode == "causal"
    nc = bass.Bass(name="attnblk")
    x_in = nc.declare_dram_parameter("x_b", [T, C], BF16, isOutput=False)
    wqkv = nc.declare_dram_parameter("wqkv", [C, 3 * CL], BF16, isOutput=False)
    bqkv = nc.declare_dram_parameter("bqkv", [3 * CL], F32, isOutput=False)
    wp = nc.declare_dram_parameter("wp", [CL, C], BF16, isOutput=False)
    if mode == "causal":
        maskd = nc.declare_dram_parameter("maskd", [128, 128], BF16, isOutput=False)
    elif mode == "full":
        maskt = nc.declare_dram_parameter("maskt", [T, T], BF16, isOutput=False)
    y_out = nc.declare_dram_parameter("y_part", [T, C], BF16, isOutput=True)

    Exp = mybir.ActivationFunctionType.Exp
    Sqrt = mybir.ActivationFunctionType.Sqrt
    Ident = mybir.ActivationFunctionType.Identity
    MULT = mybir.AluOpType.mult
    ADD = mybir.AluOpType.add
    SUBTRACT = mybir.AluOpType.subtract

    with tile.TileContext(nc) as tc:
        with (
            tc.tile_pool(name="persist", bufs=1) as pp,
            tc.tile_pool(name="small", bufs=1) as pco,
        ):
            # ---- persistent sbuf tensors
            xnT = pp.tile([128, NCC, T], BF16, tag="xnT")
            qT = pp.tile([128, 2, T], BF16, tag="qT")      # pair-stacked Q^T
            kT = pp.tile([128, 2, T], BF16, tag="kT")
            Vp = pp.tile([128, NT, HL, D + 1], BF16, tag="Vp")  # V' + ones col
            w_qkv = pp.tile([128, NCC, 3 * CL], BF16, tag="wqkv")
            w_p = pp.tile([128, 2, C], BF16, tag="w_p")
            attnT = pp.tile([128, 2, T], BF16, tag="attnT")
            eps_t = pco.tile([128, 1], F32, tag="eps")
            bq_t = pco.tile([128, 2, 1], F32, tag="bq")
            bk_t = pco.tile([128, 2, 1], F32, tag="bk")
            ones_bc = pco.tile([65, D], BF16, tag="ones_bc")
            if mode == "causal":
                maskd_t = pco.tile([128, 128], BF16, tag="maskd")

            nc.vector.memset(eps_t, LN_EPS)
            nc.vector.memset(ones_bc[:, :], 1.0)
            # ones columns for the sums row; V drains overwrite cols 0..D-1
            nc.vector.memset(Vp[:, :, :, :], 1.0)

            with (
                tc.tile_pool(name="sc_ps", bufs=3, space="PSUM") as scp,
                tc.tile_pool(name="v_ps", bufs=1, space="PSUM") as vps,
                tc.tile_pool(name="pv_ps", bufs=4, space="PSUM") as pvp,
                tc.tile_pool(name="x_pool", bufs=16) as xp,
                tc.tile_pool(name="ln_pool", bufs=6) as lnp,
                tc.tile_pool(name="xn_pool", bufs=4) as xnp,
                tc.tile_pool(name="p_pool", bufs=2 if causal else 4) as ppool,
                tc.tile_pool(name="rc_pool", bufs=4) as rcp,
                tc.tile_pool(name="m_pool", bufs=2) as mpool,
                tc.tile_pool(name="out_pool", bufs=5) as outp,
            ):
                x_tiles = {}
                def load_x(tt):
                    x_t = xp.tile([128, C], BF16, tag="x", name=f"x_{tt}")
                    nc.sync.dma_start(
                        out=x_t, in_=x_in[tt * 128 : (tt + 1) * 128, :]
                    )
                    x_tiles[tt] = x_t

                for tt in range(4):
                    load_x(tt)
                nc.sync.dma_start(
                    out=w_qkv, in_=wqkv.ap().rearrange("(cc p) n -> p cc n", p=128)
                )
                for tt in range(4, NT):
                    load_x(tt)
                nc.sync.dma_start(
                    out=w_p, in_=wp.ap().rearrange("(P p) n -> p P n", p=128)
                )
                nc.sync.dma_start(
                    out=bq_t, in_=bqkv.ap()[0:CL].rearrange("(P p) -> p P", p=128)
                )
                nc.sync.dma_start(
                    out=bk_t,
                    in_=bqkv.ap()[CL : 2 * CL].rearrange("(P p) -> p P", p=128),
                )
                if mode == "causal":
                    nc.sync.dma_start(out=maskd_t, in_=maskd[:, :])
                if vbias:
                    bv_t = pco.tile([128, CL], F32, tag="bv")
                    bv_ap = bass.AP(
                        tensor=bqkv.ap().tensor, offset=2 * CL,
                        ap=[[0, 128], [1, CL]],
                    )
                    nc.sync.dma_start(out=bv_t, in_=bv_ap)

                # ---- Phase A: LN -> bf16 xn -> DMA-xbar transpose.
                # Split into two stages emitted with a one-tile lag: the
                # DVE->Act->DVE->Act sem round-trips then overlap the
                # neighboring tile's engine work instead of blocking the
                # in-order engine queues (which would pace the pipeline at
                # the full chain latency, ~2.3us/tile).
                a_state = {}
                def phase_a_stats(tt):
                    x_t = x_tiles.pop(tt)
                    stats = lnp.tile([128, 2, 6], F32, tag="stats")
                    xg = x_t[:, :].rearrange("p (g d) -> p g d", g=2)
                    for g in range(2):
                        nc.vector.bn_stats(out=stats[:, g, :], in_=xg[:, g, :])
                    mv = lnp.tile([128, 2], F32, tag="mv")
                    nc.vector.bn_aggr(out=mv[:, :], in_=stats[:, :, :])
                    rstd = lnp.tile([128, 1], F32, tag="rstd")
                    nc.scalar.activation(
                        out=rstd[:, :], in_=mv[:, 1:2], func=Sqrt,
                        bias=eps_t[:, :], scale=1.0,
                    )
                    a_state[tt] = (x_t, mv, rstd)

                def phase_a_fin(tt):
                    if tt not in a_state:
                        return
                    x_t, mv, rstd = a_state.pop(tt)
                    nc.vector.reciprocal(out=rstd[:, :], in_=rstd[:, :])
                    nmr = lnp.tile([128, 1], F32, tag="nmr")
                    nc.gpsimd.tensor_scalar(
                        nmr[:, :], mv[:, 0:1], rstd[:, :], -1.0,
                        op0=MULT, op1=MULT,
                    )
                    # xn split Act/Pool (sbuf-only op; Pool cannot touch
                    # PSUM so it gets the sbuf work)
                    xn_t = xnp.tile([128, C], BF16, tag="xn")
                    nc.gpsimd.tensor_scalar(
                        xn_t[:, :], x_t[:, :], rstd[:, :], nmr[:, :],
                        op0=MULT, op1=ADD,
                    )
                    nc.sync.dma_start(
                        out=xnT[:, :, tt * 128 : (tt + 1) * 128],
                        in_=xn_t[:, :], transpose=True,
                    )

                # ---- Phase B: Q/K projections per 512-token group.
                # P=0 (heads 0,1) is emitted with its group so the exp stream
                # can start right after phase A; P=1 is deferred into head 1's
                # i-loop as PE filler (its drains go to Pool/DVE since Act is
                # then busy with exps).
                def emit_qk(g, which, P, on_act):
                    dest, bias_t = (qT, bq_t) if which == 0 else (kT, bk_t)
                    wcol = which * CL + P * 128
                    ps = scp.tile(
                        [128, 512], F32, tag="sc",
                        name=f"qkps_{g}_{which}_{P}",
                    )
                    for cc in range(NCC):
                        nc.tensor.matmul(
                            ps[:, :],
                            w_qkv[:, cc, wcol : wcol + 128],
                            xnT[:, cc, g * 512 : (g + 1) * 512],
                            start=(cc == 0), stop=(cc == NCC - 1),
                        )
                    if on_act:
                        nc.scalar.activation(
                            out=dest[:, P, g * 512 : (g + 1) * 512],
                            in_=ps[:, :], func=Ident,
                            bias=bias_t[:, P, :], scale=1.0,
                        )
                    else:
                        nc.vector.tensor_scalar(
                            dest[:, P, g * 512 : (g + 1) * 512],
                            ps[:, :], bias_t[:, P, :], None, op0=ADD,
                        )

                # (phase B QK emission is interleaved with phase A and early
                # head-0 score chunks below, after the helpers are defined)

                # ---- V projection (interleaved into head 0's i-loop)
                def emit_v(tt):
                    ps = vps.tile([128, 512], F32, tag="v",
                                  name=f"vps_{tt}")[:, 0:CL]
                    for cc in range(NCC):
                        nc.tensor.matmul(
                            ps[:, :],
                            xnT[:, cc, tt * 128 : (tt + 1) * 128],
                            w_qkv[:, cc, 2 * CL : 3 * CL],
                            start=(cc == 0), stop=(cc == NCC - 1),
                        )
                    psh = ps[:, :].rearrange("p (h d) -> p h d", h=HL)
                    if vbias:
                        bvh = bv_t[:, :].rearrange("p (h d) -> p h d", h=HL)
                        nc.vector.tensor_tensor(
                            out=Vp[:, tt, :, 0:D], in0=psh, in1=bvh, op=ADD
                        )
                    else:
                        nc.vector.tensor_copy(out=Vp[:, tt, :, 0:D], in_=psh)

                # ---- Phase D (emitted interleaved): output projection.
                # use_act: route drains to Act only after the exp stream ends
                npr = 0
                def emit_proj(tt, use_act):
                    nonlocal npr
                    o_t = outp.tile([128, C], BF16, tag="o", name=f"o_{tt}")
                    for n in range(2):
                        ps = scp.tile([128, 512], F32, tag="sc",
                                      name=f"prps_{tt}_{n}")
                        for P in range(2):
                            nc.tensor.matmul(
                                ps[:, :],
                                attnT[:, P, tt * 128 : (tt + 1) * 128],
                                w_p[:, P, n * 512 : (n + 1) * 512],
                                start=(P == 0), stop=(P == 1),
                            )
                        osl = o_t[:, n * 512 : (n + 1) * 512]
                        if use_act and n == 0:
                            nc.scalar.copy(out=osl, in_=ps[:, :])
                        else:
                            npr += 1
                            nc.vector.tensor_copy(out=osl, in_=ps[:, :])
                    nc.sync.dma_start(
                        out=y_out[tt * 128 : (tt + 1) * 128, :], in_=o_t[:, :]
                    )

                # ---- Phase C: attention, one flat software-pipelined slot
                # stream across all heads.  PV consumption lags scores/exp
                # production by a FULL HEAD (LAGM=16 slots) in causal mode:
                # every PV's exp dependency is then a whole head old, so PE
                # never blocks on the exp chain, and PE's surplus work (V,
                # QK-P1, proj) fills the Act-lag inside each score phase.
                LAGM = NT if causal else 2
                pv_tiles, p_tiles = {}, {}
                next_c = {}
                pending_bcast = []

                def flush_bcast():
                    """PE-side broadcast of 1/sums across partitions, emitted
                    one slot late so the DVE reciprocal has time to land."""
                    while pending_bcast:
                        h, j, rc_row = pending_bcast.pop(0)
                        P, hp = h // 2, (h % 2) * 64
                        sl = slice(512 * j, 512 * j + 512)
                        rc_ps = vps.tile([128, 512], F32, tag="v",
                                         name=f"rcps_{h}_{j}")
                        nc.tensor.matmul(
                            rc_ps[hp : hp + 64, :],
                            ones_bc[64:65, 0:64],
                            rc_row[64:65, :],
                            start=True, stop=True,
                        )
                        nc.vector.tensor_tensor(
                            out=attnT[hp : hp + 64, P, sl],
                            in0=attnT[hp : hp + 64, P, sl],
                            in1=rc_ps[hp : hp + 64, :], op=MULT,
                        )

                def emit_score_chunk(h, i):
                    """Emit ONE 512-col score chunk for (h, i).  Returns False
                    when tile i is fully emitted.  Lazily allocates the p tile
                    (exact causal width) and the head's pv psum tiles."""
                    P, hp = h // 2, (h % 2) * 64
                    base = 128 * i if causal else 0
                    width = T - base
                    c0 = next_c.get((h, i), 0)
                    if c0 >= width:
                        return False
                    if c0 == 0:
                        if causal:
                            p_t = ppool.tile([128, width], BF16, tag=f"p{i}",
                                             name=f"p_{h}_{i}")
                        else:
                            p_t = ppool.tile([128, T], BF16, tag="p",
                                             name=f"p_{h}_{i}")
                        p_tiles[(h, i)] = p_t
                    p_t = p_tiles[(h, i)]
                    w = min(512, width - c0)
                    ps = scp.tile([128, 512], F32, tag="sc",
                                  name=f"scps_{h}_{i}_{c0}")
                    nc.tensor.matmul(
                        ps[:, 0:w],
                        kT[hp : hp + D, P, i * 128 : (i + 1) * 128],
                        qT[hp : hp + D, P, base + c0 : base + c0 + w],
                        start=True, stop=True,
                    )
                    nc.scalar.activation(
                        out=p_t[:, c0 : c0 + w], in_=ps[:, 0:w],
                        func=Exp, scale=SCORE_SCALE,
                    )
                    if mode == "full":
                        m_t = mpool.tile([128, 512], BF16, tag="m")
                        nc.sync.dma_start(
                            out=m_t[:, 0:w],
                            in_=maskt[
                                i * 128 : (i + 1) * 128,
                                base + c0 : base + c0 + w,
                            ],
                        )
                        nc.vector.tensor_tensor(
                            out=p_t[:, c0 : c0 + w],
                            in0=p_t[:, c0 : c0 + w],
                            in1=m_t[:, 0:w], op=MULT,
                        )
                    if causal and c0 == 0:
                        nc.gpsimd.tensor_tensor(
                            out=p_t[:, 0:128], in0=p_t[:, 0:128],
                            in1=maskd_t[:, :], op=MULT,
                        )
                    next_c[(h, i)] = c0 + w
                    return True

                def emit_scores_all(h, i):
                    while emit_score_chunk(h, i):
                        pass

                def emit_pv(h, i):
                    P, hp = h // 2, (h % 2) * 64
                    if i == 0:
                        pv_tiles[h] = [
                            pvp.tile([65, 512], F32, tag="pv",
                                     name=f"pv_{h}_{j}")
                            for j in range(4)
                        ]
                    pv = pv_tiles[h]
                    base = 128 * i if causal else 0
                    p_t = p_tiles.pop((h, i))
                    jlo = i // 4 if causal else 0
                    for j in range(jlo, 4):
                        off = 512 * j - base   # local col in p_t
                        o0 = max(0, off)
                        skip = o0 - off        # masked lead columns
                        last = (i == 4 * j + 3) if causal else (i == NT - 1)
                        nc.tensor.matmul(
                            pv[j][:, skip : 512],
                            Vp[:, i, h, :],
                            p_t[:, o0 : off + 512],
                            start=(i == 0), stop=last,
                        )
                        if last:
                            # drain: attn^T rows to sbuf; 1/sums row via DVE
                            # (read straight from the psum sums row); the
                            # cross-partition broadcast happens on PE one
                            # slot later (flush_bcast)
                            sl = slice(512 * j, 512 * j + 512)
                            nc.vector.tensor_copy(
                                out=attnT[hp : hp + 64, P, sl],
                                in_=pv[j][0:64, :],
                            )
                            rc_row = rcp.tile([65, 512], BF16, tag="rc",
                                              name=f"rc_{h}_{j}")
                            with nc.allow_low_precision(
                                reason="f32r out is bitwise f32"
                            ):
                                nc.vector.reciprocal(
                                    out=rc_row[64:65, :], in_=pv[j][64:65, :]
                                )
                            pending_bcast.append((h, j, rc_row))

                # ---- Phases A+B (+ early head-0 score chunks in causal mode
                # so the Act exp stream starts as soon as QK-P0(g0) lands)
                for g in range(4):
                    for tt in range(4 * g, 4 * g + 4):
                        phase_a_stats(tt)
                        phase_a_fin(tt - 1)
                    phase_a_fin(4 * g + 3)
                    emit_qk(g, 0, 0, False)
                    emit_qk(g, 1, 0, False)
                    if causal:
                        cmax = 512 * (g + 1) if g < 3 else T
                        for i in range(4 * g + 4):
                            base = 128 * i
                            while True:
                                c0 = next_c.get((0, i), 0)
                                w = min(512, T - base - c0)
                                if w <= 0 or base + c0 + w > cmax:
                                    break
                                emit_score_chunk(0, i)

                # ---- main slot stream.  Causal: the whole schedule runs one
                # head-phase early (h0's scores live in phase B), so slots
                # 0-15 carry S(h1)+V, 16-31 S(h2)+PV(h0), 32-47 S(h3)+PV(h1),
                # 48-63 PV(h2), 64-79 PV(h3)+proj.
                if causal:
                    NS, SOFF, PVOFF = (HL - 1) * NT, 1, NT
                else:
                    NS, SOFF, PVOFF = HL * NT, 0, LAGM
                for g in range(HL * NT + PVOFF):
                    flush_bcast()
                    if g < NS:
                        h, i = (g // NT) + SOFF, g % NT
                        emit_scores_all(h, i)
                    if causal:
                        if g < NT:
                            emit_v(g)
                            if g in (1, 3, 5, 7):
                                emit_qk((g - 1) // 2, 0, 1, False)  # q-P1
                            elif g == 9:
                                emit_qk(0, 1, 1, False)             # k-P1 g0
                        elif g in (17, 21, 25):
                            emit_qk((g - 13) // 4, 1, 1, False)     # k-P1 g1-3
                    elif g < NS:
                        if h == 0:
                            emit_v(i)
                        elif h == 1 and i in (1, 3, 5, 7):
                            emit_qk((i - 1) // 2, 0, 1, False)
                        elif h == 1 and i == 9:
                            emit_qk(0, 1, 1, False)
                        elif h == 2 and i in (1, 5, 9):
                            emit_qk(i // 4 + 1, 1, 1, False)
                    if g >= PVOFF:
                        h2, i2 = divmod(g - PVOFF, NT)
                        emit_pv(h2, i2)
                    # proj groups interleave into the PV tail once their
                    # attn^T j-chunks are fully scaled
                    if causal and g in (70, 74, 78):
                        j = (g - 70) // 4
                        for tt in range(4 * j, 4 * j + 4):
                            emit_proj(tt, True)
                flush_bcast()

                for tt in range(12 if causal else 0, NT):
                    emit_proj(tt, True)

    _split_waits(nc, limit=1)
    return nc


def kernel(x, mask, ln_scale, ln_bias, qkv_w, qkv_b, proj_w, proj_b):
    import ml_dtypes

    bf = ml_dtypes.bfloat16
    x = np.ascontiguousarray(np.asarray(x), dtype=np.float32)
    mask2 = np.asarray(mask).reshape(T, T)
    ln_scale = np.asarray(ln_scale, dtype=np.float32)
    ln_bias = np.asarray(ln_bias, dtype=np.float32)
    qkv_w = np.asarray(qkv_w, dtype=np.float32)
    qkv_b = np.asarray(qkv_b, dtype=np.float32)
    proj_w = np.asarray(proj_w, dtype=np.float32)
    proj_b = np.asarray(proj_b, dtype=np.float32)

    # fold LayerNorm affine into the qkv projection (exact host-side algebra)
    w_eff = (ln_scale[:, None] * qkv_w).astype(np.float32)
    b_eff = (ln_bias @ qkv_w + qkv_b).astype(np.float32)

    if mask2.all():
        mode = "none"
    elif np.array_equal(mask2, np.tril(np.ones((T, T), dtype=mask2.dtype))):
        mode = "causal"
    else:
        mode = "full"

    in_maps = []
    core_ids = list(range(8))
    vbias = bool(np.any(b_eff[2 * C : 3 * C] != 0.0))
    maskt_f = None
    maskd_m = None
    if mode == "causal":
        # diag strip mask: maskT[k, q] for the 128x128 diagonal block
        maskd_m = np.ascontiguousarray(mask2[0:128, 0:128].T.astype(bf))
    elif mode == "full":
        maskt_f = np.ascontiguousarray(mask2.T.astype(bf))

    for core in core_ids:
        b = core // 4
        hs = 4 * (core % 4)
        cols_q = slice(hs * D, hs * D + CL)
        cols_k = slice(C + hs * D, C + hs * D + CL)
        cols_v = slice(2 * C + hs * D, 2 * C + hs * D + CL)
        wl = np.concatenate(
            [w_eff[:, cols_q], w_eff[:, cols_k], w_eff[:, cols_v]], axis=1
        )
        bl = np.concatenate([b_eff[cols_q], b_eff[cols_k], b_eff[cols_v]])
        im = {
            "x_b": np.ascontiguousarray(x[b].astype(bf)),
            "wqkv": np.ascontiguousarray(wl.astype(bf)),
            "bqkv": np.ascontiguousarray(bl),
            "wp": np.ascontiguousarray(
                proj_w[hs * D : hs * D + CL, :].astype(bf)
            ),
        }
        if mode == "causal":
            im["maskd"] = maskd_m
        elif mode == "full":
            im["maskt"] = maskt_f
        in_maps.append(im)

    nc = _build(mode, vbias)
    trace = bool(int(os.environ.get("KERNEL_TRACE", "0")))
    res = run_bass_kernel_spmd(nc, in_maps, core_ids=core_ids, trace=trace)
    global LAST_RESULT
    LAST_RESULT = res

    out = np.zeros((B, T, C), dtype=np.float32)
    for core in core_ids:
        out[core // 4] += res.results[core]["y_part"].astype(np.float32)
    out += proj_b[None, None, :]
    return out


if __name__ == "__main__":
    rng = np.random.default_rng(0)
    x = rng.standard_normal((B, T, C), dtype=np.float32)
    mask = np.tril(np.ones((T, T), dtype=bool))[None, None]
    ln_scale = np.ones(C, np.float32)
    ln_bias = np.zeros(C, np.float32)
    lim = float(np.sqrt(6.0 / (C + 3 * C)))
    qkv_w = rng.uniform(-lim, lim, (C, 3 * C)).astype(np.float32)
    qkv_b = np.zeros(3 * C, np.float32)
    limp = float(np.sqrt(6.0 / (C + C)))
    proj_w = rng.uniform(-limp, limp, (C, C)).astype(np.float32)
    proj_b = np.zeros(C, np.float32)
    out = kernel(x, mask, ln_scale, ln_bias, qkv_w, qkv_b, proj_w, proj_b)
    print("out", out.shape, out.dtype, np.abs(out).max())
